# revision 24
# baseline (speedup 1.0000x reference)
"""MANN LSTMCell step (scatter_memory) on 8 Trainium2 NeuronCores.

Sharding: the 32768-row memory axis is split 4096 rows/core (softmax over the
batch axis is per-mem-row, so it stays local; the memory write needs no
all-reduce in this decomposition).  Cross-core communication is one tiny
AllGather (per-batch local minima of the usage matrix, 1 KB) and one 128 KB
AllReduce (partial read vectors).  The LSTM controller is replicated on every
core.

The cosine-similarity matmul runs in fp32 (its result feeds an argmin whose
safety margin is ~4e-6); the read and write matmuls run in bf16 (their
outputs have orders-of-magnitude looser tolerances).
"""
import sys
import numpy as np

sys.path.insert(0, '/opt/trn_rl_repo')

MEM, UNITS, BATCH, IN_DIM = 32768, 128, 256, 512
N_CORES = 8
SHARD = MEM // N_CORES          # 4096 mem rows per core
T = SHARD // 128                # 32 tiles of 128 rows
CH = 4                          # tiles per DMA chunk (512 KB chunks)
NCHUNK = T // CH
USAGE_DECAY = 0.95

_state = {}


def _build():
    import concourse.bass as bass
    import concourse.bacc as bacc
    import concourse.mybir as mybir
    import concourse.tile as tile

    f32 = mybir.dt.float32
    bf16 = mybir.dt.bfloat16
    AF = mybir.ActivationFunctionType
    ALU = mybir.AluOpType

    nc = bacc.Bacc("TRN2", target_bir_lowering=False, debug=False,
                   num_devices=N_CORES)

    def din(name, shape):
        return nc.dram_tensor(name, shape, f32, kind="ExternalInput").ap()

    def dout(name, shape):
        return nc.dram_tensor(name, shape, f32, kind="ExternalOutput").ap()

    inputs_d = din("inputs", [BATCH, IN_DIM])
    r_d = din("r_tm1", [BATCH, UNITS])
    h_d = din("h_tm1", [BATCH, UNITS])
    c_d = din("c_tm1", [BATCH, UNITS])
    k_d = din("kern", [IN_DIM + UNITS, 4 * UNITS])
    rk_d = din("rec_kern", [UNITS, 4 * UNITS])
    b_d = din("bias", [1, 4 * UNITS])
    wg_d = din("wg", [1, 1])
    omw_d = din("omw", [1, 1])
    ident_d = din("ident", [128, 128])
    m_d = din("m_shard", [SHARD, UNITS])
    cwu0_d = din("c_wu_tm1", [SHARD, BATCH])
    cwlu0_d = din("c_wlu_tm1", [SHARD, BATCH])
    cwr0_d = din("c_wr_tm1", [SHARD, BATCH])

    o_cwr = dout("o_cwr", [SHARD, BATCH])
    o_cww = dout("o_cww", [SHARD, BATCH])
    o_cwu = dout("o_cwu", [SHARD, BATCH])
    o_cwlu = dout("o_cwlu", [SHARD, BATCH])
    o_mem = dout("o_mem", [SHARD, UNITS])
    o_read = dout("o_read", [BATCH, UNITS])
    o_h = dout("o_h", [BATCH, UNITS])
    o_c = dout("o_c", [BATCH, UNITS])

    with tile.TileContext(nc) as tc:
        with tc.tile_pool(name="const", bufs=1) as const, \
             tc.tile_pool(name="big", bufs=1) as big, \
             tc.tile_pool(name="wts", bufs=1) as wts, \
             tc.tile_pool(name="stream", bufs=3) as stream, \
             tc.tile_pool(name="sm", bufs=3) as sm, \
             tc.tile_pool(name="ps_read", bufs=1, space="PSUM") as ps_read, \
             tc.tile_pool(name="ps_mc", bufs=2, space="PSUM") as ps_mc, \
             tc.tile_pool(name="ps_tp", bufs=2, space="PSUM") as ps_tp, \
             tc.tile_pool(name="ps_w", bufs=2, space="PSUM") as ps_w, \
             tc.tile_pool(name="dram", bufs=1, space="DRAM") as dram:

            # ---------------- constants ----------------
            id_t = const.tile([128, 128], f32)
            nc.sync.dma_start(id_t[:], ident_d[:])
            id16 = const.tile([128, 128], bf16)
            nc.vector.tensor_copy(id16[:], id_t[:])
            id95 = const.tile([128, 128], f32)
            nc.vector.tensor_scalar(id95[:], id_t[:], USAGE_DECAY, None, ALU.mult)

            wg_raw = const.tile([1, 1], f32)
            omw_raw = const.tile([1, 1], f32)
            nc.sync.dma_start(wg_raw[:], wg_d[:])
            nc.sync.dma_start(omw_raw[:], omw_d[:])
            wg_b = const.tile([128, 1], f32)
            omw_b = const.tile([128, 1], f32)
            nc.gpsimd.partition_broadcast(wg_b[:], wg_raw[:])
            nc.gpsimd.partition_broadcast(omw_b[:], omw_raw[:])

            bias_row = const.tile([1, 512], f32)
            nc.sync.dma_start(bias_row[:], b_d[:])
            bias_b = const.tile([128, 512], f32)
            nc.gpsimd.partition_broadcast(bias_b[:], bias_row[:])

            # ---------------- LSTM controller (replicated) ----------------
            inp_sb = []
            r_sb = []
            h_sb = []
            c_sb = []
            for bh in range(2):
                t_in = wts.tile([128, 512], f32, name=f"inp{bh}")
                nc.sync.dma_start(t_in[:], inputs_d[bh * 128:(bh + 1) * 128, :])
                inp_sb.append(t_in)
                t_r = wts.tile([128, 128], f32, name=f"r{bh}")
                nc.sync.dma_start(t_r[:], r_d[bh * 128:(bh + 1) * 128, :])
                r_sb.append(t_r)
                t_h = wts.tile([128, 128], f32, name=f"h{bh}")
                nc.sync.dma_start(t_h[:], h_d[bh * 128:(bh + 1) * 128, :])
                h_sb.append(t_h)
                t_c = wts.tile([128, 128], f32, name=f"c{bh}")
                nc.sync.dma_start(t_c[:], c_d[bh * 128:(bh + 1) * 128, :])
                c_sb.append(t_c)

            k_sb = []
            for kb in range(5):
                t_k = wts.tile([128, 512], f32, name=f"k{kb}")
                nc.sync.dma_start(t_k[:], k_d[kb * 128:(kb + 1) * 128, :])
                k_sb.append(t_k)
            rk_sb = wts.tile([128, 512], f32)
            nc.sync.dma_start(rk_sb[:], rk_d[:])

            # transposed concat([inputs, r], -1): ciT[kb] is [128k, 256b]
            ciT = [wts.tile([128, 256], f32, name=f"ciT{kb}") for kb in range(5)]
            hT = wts.tile([128, 256], f32)
            for bh in range(2):
                for fb in range(4):
                    pt = ps_tp.tile([128, 128], f32, tag="tp")
                    nc.tensor.transpose(pt[:], inp_sb[bh][:, fb * 128:(fb + 1) * 128], id_t[:])
                    nc.scalar.activation(ciT[fb][:, bh * 128:(bh + 1) * 128], pt[:], AF.Copy)
                pt = ps_tp.tile([128, 128], f32, tag="tp")
                nc.tensor.transpose(pt[:], r_sb[bh][:], id_t[:])
                nc.scalar.activation(ciT[4][:, bh * 128:(bh + 1) * 128], pt[:], AF.Copy)
                pt = ps_tp.tile([128, 128], f32, tag="tp")
                nc.tensor.transpose(pt[:], h_sb[bh][:], id_t[:])
                nc.scalar.activation(hT[:, bh * 128:(bh + 1) * 128], pt[:], AF.Copy)

            h_new = []
            h16 = []
            f32r = mybir.dt.float32r
            nkey = const.tile([128, 256], f32r)
            for bh in range(2):
                zp = ps_mc.tile([128, 512], f32, tag="mc")
                for kb in range(5):
                    nc.tensor.matmul(zp[:], lhsT=ciT[kb][:, bh * 128:(bh + 1) * 128],
                                     rhs=k_sb[kb][:], start=(kb == 0), stop=False)
                nc.tensor.matmul(zp[:], lhsT=hT[:, bh * 128:(bh + 1) * 128],
                                 rhs=rk_sb[:], start=False, stop=True)
                z_sb = wts.tile([128, 512], f32, name=f"z{bh}", tag=f"inp{bh}")
                nc.vector.tensor_tensor(z_sb[:], zp[:], bias_b[:], ALU.add)

                ig = sm.tile([128, 128], f32, bufs=1)
                fg = sm.tile([128, 128], f32, bufs=1)
                gg = sm.tile([128, 128], f32, bufs=1)
                og = sm.tile([128, 128], f32, bufs=1)
                nc.scalar.activation(ig[:], z_sb[:, 0:128], AF.Sigmoid)
                nc.scalar.activation(fg[:], z_sb[:, 128:256], AF.Sigmoid)
                nc.scalar.activation(gg[:], z_sb[:, 256:384], AF.Tanh)
                nc.scalar.activation(og[:], z_sb[:, 384:512], AF.Sigmoid)
                t_ig = sm.tile([128, 128], f32, bufs=1)
                nc.vector.tensor_tensor(t_ig[:], ig[:], gg[:], ALU.mult)
                t_fc = sm.tile([128, 128], f32, bufs=1)
                nc.vector.tensor_tensor(t_fc[:], fg[:], c_sb[bh][:], ALU.mult)
                cn = const.tile([128, 128], f32, name=f"cnew{bh}")
                nc.vector.tensor_tensor(cn[:], t_fc[:], t_ig[:], ALU.add)
                tc_ = sm.tile([128, 128], f32, bufs=1)
                nc.scalar.activation(tc_[:], cn[:], AF.Tanh)
                hn = const.tile([128, 128], f32, name=f"hnew{bh}")
                nc.vector.tensor_tensor(hn[:], og[:], tc_[:], ALU.mult)
                h_new.append(hn)
                nc.sync.dma_start(o_h[bh * 128:(bh + 1) * 128, :], hn[:])
                nc.sync.dma_start(o_c[bh * 128:(bh + 1) * 128, :], cn[:])
                hb = const.tile([128, 128], bf16, name=f"h16_{bh}")
                nc.vector.tensor_copy(hb[:], hn[:])
                h16.append(hb)

                # l2-normalize rows of h (= columns of key_list)
                sq_s = sm.tile([128, 128], f32, bufs=1)
                ss = sm.tile([128, 1], f32)
                nc.scalar.activation(sq_s[:], hn[:], AF.Square, accum_out=ss[:])
                ssm = sm.tile([128, 1], f32)
                nc.vector.tensor_scalar(ssm[:], ss[:], 1e-12, None, ALU.max)
                sq2 = sm.tile([128, 1], f32)
                nc.scalar.activation(sq2[:], ssm[:], AF.Sqrt)
                rr = sm.tile([128, 1], f32)
                nc.vector.reciprocal(rr[:], sq2[:])
                nh = sm.tile([128, 128], f32, bufs=1)
                nc.vector.tensor_scalar(nh[:], hn[:], rr[:], None, ALU.mult)
                pt = ps_tp.tile([128, 128], f32, tag="tp")
                nc.tensor.transpose(pt[:], nh[:], id_t[:])
                nc.scalar.activation(nkey[:, bh * 128:(bh + 1) * 128], pt[:], AF.Copy)

            # ---------------- persistent big buffers ----------------
            m_all = big.tile([128, T * 128], f32)
            cwu_all = big.tile([128, T * 256], f32)
            w_all = big.tile([128, T * 128], f32)
            min_run = const.tile([128, 256], f32)

            read_a = ps_read.tile([128, 128], f32)
            read_b = ps_read.tile([128, 128], f32)

            # ---------------- phase A: main streaming loop ----------------
            for ci in range(NCHUNK):
                rows = slice(ci * CH * 128, (ci + 1) * CH * 128)
                csl = slice(ci * CH * 256, (ci + 1) * CH * 256)
                msl = slice(ci * CH * 128, (ci + 1) * CH * 128)
                cwu_in = stream.tile([128, CH * 256], f32, tag="cwu_in")
                cwr_in = stream.tile([128, CH * 256], f32, tag="cwr_in")
                cwlu_in = stream.tile([128, CH * 256], f32, tag="cwlu_in")
                nc.sync.dma_start(
                    cwu_in[:].rearrange("p (t b) -> p t b", b=BATCH),
                    cwu0_d[rows, :].rearrange("(t p) b -> p t b", p=128))
                nc.sync.dma_start(
                    cwr_in[:].rearrange("p (t b) -> p t b", b=BATCH),
                    cwr0_d[rows, :].rearrange("(t p) b -> p t b", p=128))
                nc.sync.dma_start(
                    cwlu_in[:].rearrange("p (t b) -> p t b", b=BATCH),
                    cwlu0_d[rows, :].rearrange("(t p) b -> p t b", p=128))
                nc.sync.dma_start(
                    m_all[:, msl].rearrange("p (t u) -> p t u", u=UNITS),
                    m_d[rows, :].rearrange("(t p) u -> p t u", p=128))

                cwr_out = stream.tile([128, CH * 256], f32, tag="cwr_out")
                cww_out = stream.tile([128, CH * 256], f32, tag="cww_out")

                # c_ww chunk: affine into cww_out, then += c_wlu_tm1 (in place)
                nc.vector.tensor_scalar(cww_out[:], cwr_in[:], wg_b[:], omw_b[:],
                                        ALU.mult, ALU.add)
                nc.vector.tensor_tensor(cww_out[:], cww_out[:], cwlu_in[:], ALU.add)
                cww16 = stream.tile([128, CH * 256], bf16, tag="cww16", bufs=1)
                nc.vector.tensor_copy(cww16[:], cww_out[:])

                # batched row-norm stats for this chunk
                ssm_c = sm.tile([128, CH], f32, tag="ssm_c")
                sqr_c = sm.tile([128, CH], f32, tag="sqr_c")
                rr_c = sm.tile([128, CH], f32, tag="rr_c")

                # pass 1: row-norm Squares (no PSUM) + write matmuls
                for t in range(CH):
                    tt = ci * CH + t
                    m_t = m_all[:, tt * 128:(tt + 1) * 128]

                    # write-matmul lhsT: bf16 c_ww transposed on TensorE.
                    # wt16a doubles as the Square scratch output (overwritten
                    # by the transpose copy below; only accum_out matters).
                    wt16a = sm.tile([128, 128], bf16, tag="wt16a", bufs=2)
                    wt16b = sm.tile([128, 128], bf16, tag="wt16b", bufs=2)
                    nc.scalar.activation(wt16a[:], m_t, AF.Square,
                                         accum_out=ssm_c[:, t:t + 1])
                    pta = ps_tp.tile([128, 128], bf16, tag="tp")
                    nc.tensor.transpose(pta[:], cww16[:, t * 256:t * 256 + 128], id16[:])
                    nc.vector.tensor_copy(wt16a[:], pta[:])
                    ptb = ps_tp.tile([128, 128], bf16, tag="tp")
                    nc.tensor.transpose(ptb[:], cww16[:, t * 256 + 128:(t + 1) * 256], id16[:])
                    nc.vector.tensor_copy(wt16b[:], ptb[:])
                    w_ps = ps_w.tile([128, 128], f32, tag="wps")
                    nc.tensor.matmul(w_ps[:], lhsT=wt16a[:], rhs=h16[0][:],
                                     start=True, stop=False)
                    nc.tensor.matmul(w_ps[:], lhsT=wt16b[:], rhs=h16[1][:],
                                     start=False, stop=True)
                    nc.vector.tensor_copy(w_all[:, tt * 128:(tt + 1) * 128], w_ps[:])

                # sqrt + reciprocal of the row norms, batched per chunk
                nc.vector.tensor_scalar(ssm_c[:], ssm_c[:], 1e-12, None, ALU.max)
                nc.scalar.activation(sqr_c[:], ssm_c[:], AF.Sqrt)
                nc.vector.reciprocal(rr_c[:], sqr_c[:])

                # pass 2: transpose m, cosine matmul, softmax
                for t in range(CH):
                    tt = ci * CH + t
                    m_t = m_all[:, tt * 128:(tt + 1) * 128]
                    bsl = slice(t * 256, (t + 1) * 256)

                    ptm = ps_tp.tile([128, 128], f32, tag="tp")
                    nc.tensor.transpose(ptm[:], m_t, id_t[:])
                    mT = sm.tile([128, 128], f32r, tag="mT")
                    nc.scalar.activation(mT[:], ptm[:], AF.Copy)

                    mc = ps_mc.tile([128, 256], f32, tag="mc")
                    nc.tensor.matmul(mc[:], lhsT=mT[:], rhs=nkey[:],
                                     start=True, stop=True)

                    et = sm.tile([128, 256], f32, tag="exp", bufs=2)
                    se = sm.tile([128, 1], f32, tag="se")
                    nc.scalar.activation(et[:], mc[:], AF.Exp,
                                         scale=rr_c[:, t:t + 1], accum_out=se[:])
                    rse = sm.tile([128, 1], f32, tag="rse")
                    nc.vector.reciprocal(rse[:], se[:])
                    nc.vector.tensor_scalar(cwr_out[:, bsl], et[:], rse[:], None,
                                            ALU.mult)

                # read matmuls (fp32)
                for t in range(CH):
                    tt = ci * CH + t
                    nc.tensor.matmul(read_a[:], lhsT=cwr_out[:, t * 256:t * 256 + 128],
                                     rhs=m_all[:, tt * 128:(tt + 1) * 128],
                                     start=(tt == 0), stop=(tt == T - 1))
                    nc.tensor.matmul(read_b[:], lhsT=cwr_out[:, t * 256 + 128:(t + 1) * 256],
                                     rhs=m_all[:, tt * 128:(tt + 1) * 128],
                                     start=(tt == 0), stop=(tt == T - 1))

                # c_wu chunk: (0.95*c_wu_tm1 + c_wr) on TensorE via identity
                # matmuls (exact; same add order as the reference), then +c_ww
                for hh in range(CH * 256 // 512):
                    hsl = slice(hh * 512, (hh + 1) * 512)
                    cwu_ps = ps_w.tile([128, 512], f32, tag="wps")
                    nc.tensor.matmul(cwu_ps[:], lhsT=id95[:], rhs=cwu_in[:, hsl],
                                     start=True, stop=False)
                    nc.tensor.matmul(cwu_ps[:], lhsT=id_t[:], rhs=cwr_out[:, hsl],
                                     start=False, stop=True)
                    nc.vector.tensor_tensor(
                        cwu_all[:, ci * CH * 256 + hh * 512:
                                ci * CH * 256 + (hh + 1) * 512],
                        cwu_ps[:], cww_out[:, hsl], ALU.add)

                # running per-batch minimum: one strided reduce over the
                # chunk's tile axis, then fold into the running min
                cwuc_v = cwu_all[:, csl].rearrange("p (t b) -> p b t", b=BATCH)
                mtree = sm.tile([128, 256], f32, tag="mtree", bufs=1)
                nc.vector.tensor_reduce(mtree[:], cwuc_v, mybir.AxisListType.X,
                                        ALU.min)
                if ci == 0:
                    nc.vector.tensor_copy(min_run[:], mtree[:])
                else:
                    nc.vector.tensor_tensor(min_run[:], min_run[:], mtree[:],
                                            ALU.min)

                # chunk outputs
                nc.sync.dma_start(
                    o_cwr[rows, :].rearrange("(t p) b -> p t b", p=128),
                    cwr_out[:].rearrange("p (t b) -> p t b", b=BATCH))
                nc.sync.dma_start(
                    o_cww[rows, :].rearrange("(t p) b -> p t b", p=128),
                    cww_out[:].rearrange("p (t b) -> p t b", b=BATCH))
            # ---------------- global min via AllGather ----------------
            negmin = const.tile([128, 256], f32)
            nc.vector.tensor_scalar(negmin[:], min_run[:], -1.0, None, ALU.mult)
            negred = const.tile([128, 256], f32)
            nc.gpsimd.partition_all_reduce(negred[:], negmin[:], channels=128,
                                           reduce_op=_reduce_max())
            mn_in = dram.tile([1, 256], f32)
            mn_out = dram.tile([N_CORES, 256], f32, addr_space="Shared")
            nc.sync.dma_start(mn_in[:], negred[0:1, :])
            cc_ag = nc.gpsimd.collective_compute(
                "AllGather", mybir.AluOpType.bypass,
                replica_groups=[list(range(N_CORES))],
                ins=[mn_in[:].opt()], outs=[mn_out[:].opt()])
            # c_wu output writeback deferred to here: it reads the persistent
            # cwu_all buffer and fills the AllGather skew window with DMA work
            for ci in range(NCHUNK):
                rows = slice(ci * CH * 128, (ci + 1) * CH * 128)
                csl = slice(ci * CH * 256, (ci + 1) * CH * 256)
                nc.sync.dma_start(
                    o_cwu[rows, :].rearrange("(t p) b -> p t b", p=128),
                    cwu_all[:, csl].rearrange("p (t b) -> p t b", b=BATCH))
            negall = const.tile([N_CORES, 256], f32)
            nc.sync.dma_start(negall[:], mn_out[:])
            negall_r = const.tile([N_CORES, 256], f32)
            nc.gpsimd.partition_all_reduce(negall_r[:], negall[:], channels=N_CORES,
                                           reduce_op=_reduce_max())
            gmin_row = const.tile([1, 256], f32)
            nc.vector.tensor_scalar(gmin_row[:], negall_r[0:1, :], -1.0, None,
                                    ALU.mult)
            gmin_b = const.tile([128, 256], f32)
            nc.gpsimd.partition_broadcast(gmin_b[:], gmin_row[:])
            gmin_rep = gmin_b[:].rearrange("p (x b) -> p x b", x=1) \
                                .broadcast_to([128, CH, 256])

            # ---------------- phase C: c_wlu + memory write ----------------
            for ci in range(NCHUNK):
                rows = slice(ci * CH * 128, (ci + 1) * CH * 128)
                csl = slice(ci * CH * 256, (ci + 1) * CH * 256)
                msl = slice(ci * CH * 128, (ci + 1) * CH * 128)
                # reuse phase-A stream slots (those tags are dead by now)
                cwlu_out = stream.tile([128, CH * 256], f32, tag="cwu_in")
                memc = stream.tile([128, CH * 128], f32, tag="cwr_out")
                nc.vector.tensor_tensor(
                    cwlu_out[:].rearrange("p (t b) -> p t b", b=BATCH),
                    cwu_all[:, csl].rearrange("p (t b) -> p t b", b=BATCH),
                    gmin_rep, ALU.is_le)
                cnt_c = sm.tile([128, CH], f32, tag="cnt_c")
                nc.vector.tensor_reduce(
                    cnt_c[:], cwlu_out[:].rearrange("p (t b) -> p t b", b=BATCH),
                    mybir.AxisListType.X, ALU.add)
                scl_c = sm.tile([128, CH], f32, tag="scl_c")
                nc.vector.tensor_scalar(scl_c[:], cnt_c[:], -1.0, float(BATCH),
                                        ALU.mult, ALU.add)
                for t in range(CH):
                    tt = ci * CH + t
                    nc.scalar.activation(memc[:, t * 128:(t + 1) * 128],
                                         m_all[:, tt * 128:(tt + 1) * 128],
                                         AF.Identity, scale=scl_c[:, t:t + 1])
                nc.vector.tensor_tensor(memc[:], memc[:], w_all[:, msl], ALU.add)
                nc.sync.dma_start(
                    o_cwlu[rows, :].rearrange("(t p) b -> p t b", p=128),
                    cwlu_out[:].rearrange("p (t b) -> p t b", b=BATCH))
                nc.sync.dma_start(
                    o_mem[rows, :].rearrange("(t p) u -> p t u", p=128),
                    memc[:].rearrange("p (t u) -> p t u", u=UNITS))

            # ---------------- read partial all-reduce (off critical path) ----
            read_sb = const.tile([128, 256], f32)
            nc.vector.tensor_copy(read_sb[:, 0:128], read_a[:])
            nc.vector.tensor_copy(read_sb[:, 128:256], read_b[:])
            rd_in = dram.tile([BATCH, UNITS], f32)
            rd_out = dram.tile([BATCH, UNITS], f32, addr_space="Shared")
            nc.sync.dma_start(rd_in[0:128, :], read_sb[:, 0:128])
            nc.sync.dma_start(rd_in[128:256, :], read_sb[:, 128:256])
            cc_ar = nc.gpsimd.collective_compute(
                "AllReduce", mybir.AluOpType.add,
                replica_groups=[list(range(N_CORES))],
                ins=[rd_in[:].opt()], outs=[rd_out[:].opt()])
            from concourse.tile_rust import add_dep_helper
            add_dep_helper(cc_ar.ins, cc_ag.ins, sync=True,
                           reason="AllGather feeds the critical path; run it first")
            nc.sync.dma_start(o_read[:], rd_out[:])

    nc.compile()
    return nc


def _reduce_max():
    from concourse import bass_isa
    return bass_isa.ReduceOp.max


def _ensure_built():
    if "nc" not in _state:
        _state["nc"] = _build()
    return _state["nc"]


def kernel(**inputs):
    from concourse import bass_utils

    nc = _ensure_built()

    inp = {k: np.asarray(v, dtype=np.float32) for k, v in inputs.items()}
    wg = 1.0 / (1.0 + np.exp(-inp["write_gate"].astype(np.float64)))
    wg32 = wg.astype(np.float32).reshape(1, 1)
    omw32 = (1.0 - wg32).astype(np.float32)

    shared = {
        "inputs": inp["inputs"],
        "r_tm1": inp["r_tm1"],
        "h_tm1": inp["h_tm1"],
        "c_tm1": inp["c_tm1"],
        "kern": inp["kernel"],
        "rec_kern": inp["rec_kernel"],
        "bias": inp["bias"].reshape(1, 4 * UNITS),
        "wg": wg32,
        "omw": omw32,
        "ident": np.eye(128, dtype=np.float32),
    }
    in_maps = []
    for c in range(N_CORES):
        rows = slice(c * SHARD, (c + 1) * SHARD)
        m = dict(shared)
        m["m_shard"] = np.ascontiguousarray(inp["m_tm1"][rows])
        m["c_wu_tm1"] = np.ascontiguousarray(inp["c_wu_tm1"][rows])
        m["c_wlu_tm1"] = np.ascontiguousarray(inp["c_wlu_tm1"][rows])
        m["c_wr_tm1"] = np.ascontiguousarray(inp["c_wr_tm1"][rows])
        in_maps.append(m)

    res = bass_utils.run_bass_kernel_spmd(
        nc, in_maps, core_ids=list(range(N_CORES)),
        trace=bool(_state.get("trace", False)))
    _state["last_result"] = res
    r = res.results

    read = r[0]["o_read"]
    h = r[0]["o_h"]
    c = r[0]["o_c"]
    memory = np.concatenate([r[i]["o_mem"] for i in range(N_CORES)], axis=0)
    c_wu = np.concatenate([r[i]["o_cwu"] for i in range(N_CORES)], axis=0)
    c_wlu = np.concatenate([r[i]["o_cwlu"] for i in range(N_CORES)], axis=0)
    c_wr = np.concatenate([r[i]["o_cwr"] for i in range(N_CORES)], axis=0)
    c_ww = np.concatenate([r[i]["o_cww"] for i in range(N_CORES)], axis=0)
    return read, memory, c_wu, c_wlu, c_wr, c_ww, h, c


# revision 25
# speedup vs baseline: 1.0655x; 1.0655x over previous
"""MANN LSTMCell step (scatter_memory) on 8 Trainium2 NeuronCores.

Sharding: the 32768-row memory axis is split 4096 rows/core (softmax over the
batch axis is per-mem-row, so it stays local; the memory write needs no
all-reduce in this decomposition).  Cross-core communication is one tiny
AllGather (per-batch local minima of the usage matrix, 1 KB) and one 128 KB
AllReduce (partial read vectors).  The LSTM controller is replicated on every
core.

The cosine-similarity matmul runs in fp32 (its result feeds an argmin whose
safety margin is ~4e-6); the read and write matmuls run in bf16 (their
outputs have orders-of-magnitude looser tolerances).
"""
import sys
import numpy as np

sys.path.insert(0, '/opt/trn_rl_repo')

MEM, UNITS, BATCH, IN_DIM = 32768, 128, 256, 512
N_CORES = 8
SHARD = MEM // N_CORES          # 4096 mem rows per core
T = SHARD // 128                # 32 tiles of 128 rows
CH = 4                          # tiles per DMA chunk (512 KB chunks)
NCHUNK = T // CH
USAGE_DECAY = 0.95

_state = {}


def _build():
    import concourse.bass as bass
    import concourse.bacc as bacc
    import concourse.mybir as mybir
    import concourse.tile as tile

    f32 = mybir.dt.float32
    bf16 = mybir.dt.bfloat16
    AF = mybir.ActivationFunctionType
    ALU = mybir.AluOpType

    nc = bacc.Bacc("TRN2", target_bir_lowering=False, debug=False,
                   num_devices=N_CORES)

    def din(name, shape):
        return nc.dram_tensor(name, shape, f32, kind="ExternalInput").ap()

    def dout(name, shape):
        return nc.dram_tensor(name, shape, f32, kind="ExternalOutput").ap()

    inputs_d = din("inputs", [BATCH, IN_DIM])
    r_d = din("r_tm1", [BATCH, UNITS])
    h_d = din("h_tm1", [BATCH, UNITS])
    c_d = din("c_tm1", [BATCH, UNITS])
    k_d = din("kern", [IN_DIM + UNITS, 4 * UNITS])
    rk_d = din("rec_kern", [UNITS, 4 * UNITS])
    b_d = din("bias", [1, 4 * UNITS])
    wg_d = din("wg", [1, 1])
    omw_d = din("omw", [1, 1])
    ident_d = din("ident", [128, 128])
    m_d = din("m_shard", [SHARD, UNITS])
    cwu0_d = din("c_wu_tm1", [SHARD, BATCH])
    cwlu0_d = din("c_wlu_tm1", [SHARD, BATCH])
    cwr0_d = din("c_wr_tm1", [SHARD, BATCH])

    o_cwr = dout("o_cwr", [SHARD, BATCH])
    o_cww = dout("o_cww", [SHARD, BATCH])
    o_cwu = dout("o_cwu", [SHARD, BATCH])
    o_cwlu = dout("o_cwlu", [SHARD, BATCH])
    o_mem = dout("o_mem", [SHARD, UNITS])
    o_read = dout("o_read", [BATCH, UNITS])
    o_h = dout("o_h", [BATCH, UNITS])
    o_c = dout("o_c", [BATCH, UNITS])

    with tile.TileContext(nc) as tc:
        with tc.tile_pool(name="const", bufs=1) as const, \
             tc.tile_pool(name="big", bufs=1) as big, \
             tc.tile_pool(name="wts", bufs=1) as wts, \
             tc.tile_pool(name="stream", bufs=3) as stream, \
             tc.tile_pool(name="sm", bufs=3) as sm, \
             tc.tile_pool(name="ps_read", bufs=1, space="PSUM") as ps_read, \
             tc.tile_pool(name="ps_mc", bufs=2, space="PSUM") as ps_mc, \
             tc.tile_pool(name="ps_tp", bufs=2, space="PSUM") as ps_tp, \
             tc.tile_pool(name="ps_w", bufs=2, space="PSUM") as ps_w, \
             tc.tile_pool(name="dram", bufs=1, space="DRAM") as dram:

            # ---------------- constants ----------------
            id_t = const.tile([128, 128], f32)
            nc.sync.dma_start(id_t[:], ident_d[:])
            id16 = const.tile([128, 128], bf16)
            nc.vector.tensor_copy(id16[:], id_t[:])
            id95 = const.tile([128, 128], f32)
            nc.vector.tensor_scalar(id95[:], id_t[:], USAGE_DECAY, None, ALU.mult)

            wg_raw = const.tile([1, 1], f32)
            omw_raw = const.tile([1, 1], f32)
            nc.sync.dma_start(wg_raw[:], wg_d[:])
            nc.sync.dma_start(omw_raw[:], omw_d[:])
            wg_b = const.tile([128, 1], f32)
            omw_b = const.tile([128, 1], f32)
            nc.gpsimd.partition_broadcast(wg_b[:], wg_raw[:])
            nc.gpsimd.partition_broadcast(omw_b[:], omw_raw[:])

            bias_row = const.tile([1, 512], f32)
            nc.sync.dma_start(bias_row[:], b_d[:])
            bias_b = const.tile([128, 512], f32)
            nc.gpsimd.partition_broadcast(bias_b[:], bias_row[:])

            # ---------------- LSTM controller (replicated) ----------------
            inp_sb = []
            r_sb = []
            h_sb = []
            c_sb = []
            for bh in range(2):
                t_in = wts.tile([128, 512], f32, name=f"inp{bh}")
                nc.sync.dma_start(t_in[:], inputs_d[bh * 128:(bh + 1) * 128, :])
                inp_sb.append(t_in)
                t_r = wts.tile([128, 128], f32, name=f"r{bh}")
                nc.sync.dma_start(t_r[:], r_d[bh * 128:(bh + 1) * 128, :])
                r_sb.append(t_r)
                t_h = wts.tile([128, 128], f32, name=f"h{bh}")
                nc.sync.dma_start(t_h[:], h_d[bh * 128:(bh + 1) * 128, :])
                h_sb.append(t_h)
                t_c = wts.tile([128, 128], f32, name=f"c{bh}")
                nc.sync.dma_start(t_c[:], c_d[bh * 128:(bh + 1) * 128, :])
                c_sb.append(t_c)

            k_sb = []
            for kb in range(5):
                t_k = wts.tile([128, 512], f32, name=f"k{kb}")
                nc.sync.dma_start(t_k[:], k_d[kb * 128:(kb + 1) * 128, :])
                k_sb.append(t_k)
            rk_sb = wts.tile([128, 512], f32)
            nc.sync.dma_start(rk_sb[:], rk_d[:])

            # transposed concat([inputs, r], -1): ciT[kb] is [128k, 256b]
            ciT = [wts.tile([128, 256], f32, name=f"ciT{kb}") for kb in range(5)]
            hT = wts.tile([128, 256], f32)
            for bh in range(2):
                for fb in range(4):
                    pt = ps_tp.tile([128, 128], f32, tag="tp")
                    nc.tensor.transpose(pt[:], inp_sb[bh][:, fb * 128:(fb + 1) * 128], id_t[:])
                    nc.scalar.activation(ciT[fb][:, bh * 128:(bh + 1) * 128], pt[:], AF.Copy)
                pt = ps_tp.tile([128, 128], f32, tag="tp")
                nc.tensor.transpose(pt[:], r_sb[bh][:], id_t[:])
                nc.scalar.activation(ciT[4][:, bh * 128:(bh + 1) * 128], pt[:], AF.Copy)
                pt = ps_tp.tile([128, 128], f32, tag="tp")
                nc.tensor.transpose(pt[:], h_sb[bh][:], id_t[:])
                nc.scalar.activation(hT[:, bh * 128:(bh + 1) * 128], pt[:], AF.Copy)

            h_new = []
            h16 = []
            f32r = mybir.dt.float32r
            nkey = const.tile([128, 256], f32r)
            for bh in range(2):
                zp = ps_mc.tile([128, 512], f32, tag="mc")
                for kb in range(5):
                    nc.tensor.matmul(zp[:], lhsT=ciT[kb][:, bh * 128:(bh + 1) * 128],
                                     rhs=k_sb[kb][:], start=(kb == 0), stop=False)
                nc.tensor.matmul(zp[:], lhsT=hT[:, bh * 128:(bh + 1) * 128],
                                 rhs=rk_sb[:], start=False, stop=True)
                z_sb = wts.tile([128, 512], f32, name=f"z{bh}", tag=f"inp{bh}")
                nc.vector.tensor_tensor(z_sb[:], zp[:], bias_b[:], ALU.add)

                ig = sm.tile([128, 128], f32, bufs=1)
                fg = sm.tile([128, 128], f32, bufs=1)
                gg = sm.tile([128, 128], f32, bufs=1)
                og = sm.tile([128, 128], f32, bufs=1)
                nc.scalar.activation(ig[:], z_sb[:, 0:128], AF.Sigmoid)
                nc.scalar.activation(fg[:], z_sb[:, 128:256], AF.Sigmoid)
                nc.scalar.activation(gg[:], z_sb[:, 256:384], AF.Tanh)
                nc.scalar.activation(og[:], z_sb[:, 384:512], AF.Sigmoid)
                t_ig = sm.tile([128, 128], f32, bufs=1)
                nc.vector.tensor_tensor(t_ig[:], ig[:], gg[:], ALU.mult)
                t_fc = sm.tile([128, 128], f32, bufs=1)
                nc.vector.tensor_tensor(t_fc[:], fg[:], c_sb[bh][:], ALU.mult)
                cn = const.tile([128, 128], f32, name=f"cnew{bh}")
                nc.vector.tensor_tensor(cn[:], t_fc[:], t_ig[:], ALU.add)
                tc_ = sm.tile([128, 128], f32, bufs=1)
                nc.scalar.activation(tc_[:], cn[:], AF.Tanh)
                hn = const.tile([128, 128], f32, name=f"hnew{bh}")
                nc.vector.tensor_tensor(hn[:], og[:], tc_[:], ALU.mult)
                h_new.append(hn)
                nc.sync.dma_start(o_h[bh * 128:(bh + 1) * 128, :], hn[:])
                nc.sync.dma_start(o_c[bh * 128:(bh + 1) * 128, :], cn[:])
                hb = const.tile([128, 128], bf16, name=f"h16_{bh}")
                nc.vector.tensor_copy(hb[:], hn[:])
                h16.append(hb)

                # l2-normalize rows of h (= columns of key_list)
                sq_s = sm.tile([128, 128], f32, bufs=1)
                ss = sm.tile([128, 1], f32)
                nc.scalar.activation(sq_s[:], hn[:], AF.Square, accum_out=ss[:])
                ssm = sm.tile([128, 1], f32)
                nc.vector.tensor_scalar(ssm[:], ss[:], 1e-12, None, ALU.max)
                sq2 = sm.tile([128, 1], f32)
                nc.scalar.activation(sq2[:], ssm[:], AF.Sqrt)
                rr = sm.tile([128, 1], f32)
                nc.vector.reciprocal(rr[:], sq2[:])
                nh = sm.tile([128, 128], f32, bufs=1)
                nc.vector.tensor_scalar(nh[:], hn[:], rr[:], None, ALU.mult)
                pt = ps_tp.tile([128, 128], f32, tag="tp")
                nc.tensor.transpose(pt[:], nh[:], id_t[:])
                nc.scalar.activation(nkey[:, bh * 128:(bh + 1) * 128], pt[:], AF.Copy)

            # ---------------- persistent big buffers ----------------
            m_all = big.tile([128, T * 128], f32)
            cwu_all = big.tile([128, T * 256], f32)
            w_all = big.tile([128, T * 128], f32)
            min_run = const.tile([128, 256], f32)

            read_a = ps_read.tile([128, 128], f32)
            read_b = ps_read.tile([128, 128], f32)

            # ---------------- phase A: main streaming loop ----------------
            for ci in range(NCHUNK):
                rows = slice(ci * CH * 128, (ci + 1) * CH * 128)
                csl = slice(ci * CH * 256, (ci + 1) * CH * 256)
                msl = slice(ci * CH * 128, (ci + 1) * CH * 128)
                cwu_in = stream.tile([128, CH * 256], f32, tag="cwu_in")
                cwr_in = stream.tile([128, CH * 256], f32, tag="cwr_in")
                cwlu_in = stream.tile([128, CH * 256], f32, tag="cwlu_in")
                nc.sync.dma_start(
                    cwu_in[:].rearrange("p (t b) -> p t b", b=BATCH),
                    cwu0_d[rows, :].rearrange("(t p) b -> p t b", p=128))
                nc.sync.dma_start(
                    cwr_in[:].rearrange("p (t b) -> p t b", b=BATCH),
                    cwr0_d[rows, :].rearrange("(t p) b -> p t b", p=128))
                nc.sync.dma_start(
                    cwlu_in[:].rearrange("p (t b) -> p t b", b=BATCH),
                    cwlu0_d[rows, :].rearrange("(t p) b -> p t b", p=128))
                nc.sync.dma_start(
                    m_all[:, msl].rearrange("p (t u) -> p t u", u=UNITS),
                    m_d[rows, :].rearrange("(t p) u -> p t u", p=128))

                cwr_out = stream.tile([128, CH * 256], f32, tag="cwr_out")
                cww_out = stream.tile([128, CH * 256], f32, tag="cww_out")

                # c_ww chunk: affine into cww_out, then += c_wlu_tm1 (in place)
                nc.vector.tensor_scalar(cww_out[:], cwr_in[:], wg_b[:], omw_b[:],
                                        ALU.mult, ALU.add)
                nc.vector.tensor_tensor(cww_out[:], cww_out[:], cwlu_in[:], ALU.add)
                cww16 = stream.tile([128, CH * 256], bf16, tag="cww16", bufs=1)
                nc.vector.tensor_copy(cww16[:], cww_out[:])

                # batched row-norm stats for this chunk
                ssm_c = sm.tile([128, CH], f32, tag="ssm_c")
                sqr_c = sm.tile([128, CH], f32, tag="sqr_c")
                rr_c = sm.tile([128, CH], f32, tag="rr_c")

                # pass 1: row-norm Squares (no PSUM) + write matmuls
                for t in range(CH):
                    tt = ci * CH + t
                    m_t = m_all[:, tt * 128:(tt + 1) * 128]

                    # write-matmul lhsT: bf16 c_ww transposed on TensorE.
                    # wt16a doubles as the Square scratch output (overwritten
                    # by the transpose copy below; only accum_out matters).
                    wt16a = sm.tile([128, 128], bf16, tag="wt16a", bufs=2)
                    wt16b = sm.tile([128, 128], bf16, tag="wt16b", bufs=2)
                    nc.scalar.activation(wt16a[:], m_t, AF.Square,
                                         accum_out=ssm_c[:, t:t + 1])
                    pta = ps_tp.tile([128, 128], bf16, tag="tp")
                    nc.tensor.transpose(pta[:], cww16[:, t * 256:t * 256 + 128], id16[:])
                    nc.vector.tensor_copy(wt16a[:], pta[:])
                    ptb = ps_tp.tile([128, 128], bf16, tag="tp")
                    nc.tensor.transpose(ptb[:], cww16[:, t * 256 + 128:(t + 1) * 256], id16[:])
                    nc.vector.tensor_copy(wt16b[:], ptb[:])
                    w_ps = ps_w.tile([128, 128], f32, tag="wps")
                    nc.tensor.matmul(w_ps[:], lhsT=wt16a[:], rhs=h16[0][:],
                                     start=True, stop=False)
                    nc.tensor.matmul(w_ps[:], lhsT=wt16b[:], rhs=h16[1][:],
                                     start=False, stop=True)
                    nc.vector.tensor_copy(w_all[:, tt * 128:(tt + 1) * 128], w_ps[:])

                # sqrt + reciprocal of the row norms, batched per chunk
                nc.vector.tensor_scalar(ssm_c[:], ssm_c[:], 1e-12, None, ALU.max)
                nc.scalar.activation(sqr_c[:], ssm_c[:], AF.Sqrt)
                nc.vector.reciprocal(rr_c[:], sqr_c[:])

                # pass 2: transpose m, cosine matmul, softmax
                for t in range(CH):
                    tt = ci * CH + t
                    m_t = m_all[:, tt * 128:(tt + 1) * 128]
                    bsl = slice(t * 256, (t + 1) * 256)

                    ptm = ps_tp.tile([128, 128], f32, tag="tp")
                    nc.tensor.transpose(ptm[:], m_t, id_t[:])
                    mT = sm.tile([128, 128], f32r, tag="mT")
                    nc.scalar.activation(mT[:], ptm[:], AF.Copy)

                    mc = ps_mc.tile([128, 256], f32, tag="mc")
                    nc.tensor.matmul(mc[:], lhsT=mT[:], rhs=nkey[:],
                                     start=True, stop=True)

                    et = sm.tile([128, 256], f32, tag="exp", bufs=2)
                    se = sm.tile([128, 1], f32, tag="se")
                    nc.scalar.activation(et[:], mc[:], AF.Exp,
                                         scale=rr_c[:, t:t + 1], accum_out=se[:])
                    rse = sm.tile([128, 1], f32, tag="rse")
                    nc.vector.reciprocal(rse[:], se[:])
                    nc.vector.tensor_scalar(cwr_out[:, bsl], et[:], rse[:], None,
                                            ALU.mult)

                # read matmuls (fp32)
                for t in range(CH):
                    tt = ci * CH + t
                    nc.tensor.matmul(read_a[:], lhsT=cwr_out[:, t * 256:t * 256 + 128],
                                     rhs=m_all[:, tt * 128:(tt + 1) * 128],
                                     start=(tt == 0), stop=(tt == T - 1))
                    nc.tensor.matmul(read_b[:], lhsT=cwr_out[:, t * 256 + 128:(t + 1) * 256],
                                     rhs=m_all[:, tt * 128:(tt + 1) * 128],
                                     start=(tt == 0), stop=(tt == T - 1))

                # c_wu chunk: 0.95*c_wu_tm1 + c_wr + c_ww  (in place in cwu_in)
                nc.vector.tensor_scalar(cwu_in[:], cwu_in[:], USAGE_DECAY, None,
                                        ALU.mult)
                nc.vector.tensor_tensor(cwu_in[:], cwu_in[:], cwr_out[:], ALU.add)
                nc.vector.tensor_tensor(cwu_all[:, csl], cwu_in[:], cww_out[:],
                                        ALU.add)

                # running per-batch minimum: one strided reduce over the
                # chunk's tile axis, then fold into the running min
                cwuc_v = cwu_all[:, csl].rearrange("p (t b) -> p b t", b=BATCH)
                mtree = sm.tile([128, 256], f32, tag="mtree", bufs=1)
                nc.vector.tensor_reduce(mtree[:], cwuc_v, mybir.AxisListType.X,
                                        ALU.min)
                if ci == 0:
                    nc.vector.tensor_copy(min_run[:], mtree[:])
                else:
                    nc.vector.tensor_tensor(min_run[:], min_run[:], mtree[:],
                                            ALU.min)

                # chunk outputs
                nc.sync.dma_start(
                    o_cwr[rows, :].rearrange("(t p) b -> p t b", p=128),
                    cwr_out[:].rearrange("p (t b) -> p t b", b=BATCH))
                nc.sync.dma_start(
                    o_cww[rows, :].rearrange("(t p) b -> p t b", p=128),
                    cww_out[:].rearrange("p (t b) -> p t b", b=BATCH))
            # ---------------- global min via AllGather ----------------
            negmin = const.tile([128, 256], f32)
            nc.vector.tensor_scalar(negmin[:], min_run[:], -1.0, None, ALU.mult)
            negred = const.tile([128, 256], f32)
            nc.gpsimd.partition_all_reduce(negred[:], negmin[:], channels=128,
                                           reduce_op=_reduce_max())
            mn_in = dram.tile([1, 256], f32)
            mn_out = dram.tile([N_CORES, 256], f32, addr_space="Shared")
            nc.sync.dma_start(mn_in[:], negred[0:1, :])
            cc_ag = nc.gpsimd.collective_compute(
                "AllGather", mybir.AluOpType.bypass,
                replica_groups=[list(range(N_CORES))],
                ins=[mn_in[:].opt()], outs=[mn_out[:].opt()])
            # c_wu output writeback deferred to here: it reads the persistent
            # cwu_all buffer and fills the AllGather skew window with DMA work
            for ci in range(NCHUNK):
                rows = slice(ci * CH * 128, (ci + 1) * CH * 128)
                csl = slice(ci * CH * 256, (ci + 1) * CH * 256)
                nc.sync.dma_start(
                    o_cwu[rows, :].rearrange("(t p) b -> p t b", p=128),
                    cwu_all[:, csl].rearrange("p (t b) -> p t b", b=BATCH))
            negall = const.tile([N_CORES, 256], f32)
            nc.sync.dma_start(negall[:], mn_out[:])
            negall_r = const.tile([N_CORES, 256], f32)
            nc.gpsimd.partition_all_reduce(negall_r[:], negall[:], channels=N_CORES,
                                           reduce_op=_reduce_max())
            gmin_row = const.tile([1, 256], f32)
            nc.vector.tensor_scalar(gmin_row[:], negall_r[0:1, :], -1.0, None,
                                    ALU.mult)
            gmin_b = const.tile([128, 256], f32)
            nc.gpsimd.partition_broadcast(gmin_b[:], gmin_row[:])
            gmin_rep = gmin_b[:].rearrange("p (x b) -> p x b", x=1) \
                                .broadcast_to([128, CH, 256])

            # ---------------- phase C: c_wlu + memory write ----------------
            for ci in range(NCHUNK):
                rows = slice(ci * CH * 128, (ci + 1) * CH * 128)
                csl = slice(ci * CH * 256, (ci + 1) * CH * 256)
                msl = slice(ci * CH * 128, (ci + 1) * CH * 128)
                # reuse phase-A stream slots (those tags are dead by now)
                cwlu_out = stream.tile([128, CH * 256], f32, tag="cwu_in")
                memc = stream.tile([128, CH * 128], f32, tag="cwr_out")
                nc.vector.tensor_tensor(
                    cwlu_out[:].rearrange("p (t b) -> p t b", b=BATCH),
                    cwu_all[:, csl].rearrange("p (t b) -> p t b", b=BATCH),
                    gmin_rep, ALU.is_le)
                cnt_c = sm.tile([128, CH], f32, tag="cnt_c")
                nc.vector.tensor_reduce(
                    cnt_c[:], cwlu_out[:].rearrange("p (t b) -> p t b", b=BATCH),
                    mybir.AxisListType.X, ALU.add)
                scl_c = sm.tile([128, CH], f32, tag="scl_c")
                nc.vector.tensor_scalar(scl_c[:], cnt_c[:], -1.0, float(BATCH),
                                        ALU.mult, ALU.add)
                for t in range(CH):
                    tt = ci * CH + t
                    nc.scalar.activation(memc[:, t * 128:(t + 1) * 128],
                                         m_all[:, tt * 128:(tt + 1) * 128],
                                         AF.Identity, scale=scl_c[:, t:t + 1])
                nc.vector.tensor_tensor(memc[:], memc[:], w_all[:, msl], ALU.add)
                nc.sync.dma_start(
                    o_cwlu[rows, :].rearrange("(t p) b -> p t b", p=128),
                    cwlu_out[:].rearrange("p (t b) -> p t b", b=BATCH))
                nc.sync.dma_start(
                    o_mem[rows, :].rearrange("(t p) u -> p t u", p=128),
                    memc[:].rearrange("p (t u) -> p t u", u=UNITS))

            # ---------------- read partial all-reduce (off critical path) ----
            read_sb = const.tile([128, 256], f32)
            nc.vector.tensor_copy(read_sb[:, 0:128], read_a[:])
            nc.vector.tensor_copy(read_sb[:, 128:256], read_b[:])
            rd_in = dram.tile([BATCH, UNITS], f32)
            rd_out = dram.tile([BATCH, UNITS], f32, addr_space="Shared")
            nc.sync.dma_start(rd_in[0:128, :], read_sb[:, 0:128])
            nc.sync.dma_start(rd_in[128:256, :], read_sb[:, 128:256])
            cc_ar = nc.gpsimd.collective_compute(
                "AllReduce", mybir.AluOpType.add,
                replica_groups=[list(range(N_CORES))],
                ins=[rd_in[:].opt()], outs=[rd_out[:].opt()])
            from concourse.tile_rust import add_dep_helper
            add_dep_helper(cc_ar.ins, cc_ag.ins, sync=True,
                           reason="AllGather feeds the critical path; run it first")
            nc.sync.dma_start(o_read[:], rd_out[:])

    nc.compile()
    return nc


def _reduce_max():
    from concourse import bass_isa
    return bass_isa.ReduceOp.max


def _ensure_built():
    if "nc" not in _state:
        _state["nc"] = _build()
    return _state["nc"]


def kernel(**inputs):
    from concourse import bass_utils

    nc = _ensure_built()

    inp = {k: np.asarray(v, dtype=np.float32) for k, v in inputs.items()}
    wg = 1.0 / (1.0 + np.exp(-inp["write_gate"].astype(np.float64)))
    wg32 = wg.astype(np.float32).reshape(1, 1)
    omw32 = (1.0 - wg32).astype(np.float32)

    shared = {
        "inputs": inp["inputs"],
        "r_tm1": inp["r_tm1"],
        "h_tm1": inp["h_tm1"],
        "c_tm1": inp["c_tm1"],
        "kern": inp["kernel"],
        "rec_kern": inp["rec_kernel"],
        "bias": inp["bias"].reshape(1, 4 * UNITS),
        "wg": wg32,
        "omw": omw32,
        "ident": np.eye(128, dtype=np.float32),
    }
    in_maps = []
    for c in range(N_CORES):
        rows = slice(c * SHARD, (c + 1) * SHARD)
        m = dict(shared)
        m["m_shard"] = np.ascontiguousarray(inp["m_tm1"][rows])
        m["c_wu_tm1"] = np.ascontiguousarray(inp["c_wu_tm1"][rows])
        m["c_wlu_tm1"] = np.ascontiguousarray(inp["c_wlu_tm1"][rows])
        m["c_wr_tm1"] = np.ascontiguousarray(inp["c_wr_tm1"][rows])
        in_maps.append(m)

    res = bass_utils.run_bass_kernel_spmd(
        nc, in_maps, core_ids=list(range(N_CORES)),
        trace=bool(_state.get("trace", False)))
    _state["last_result"] = res
    r = res.results

    read = r[0]["o_read"]
    h = r[0]["o_h"]
    c = r[0]["o_c"]
    memory = np.concatenate([r[i]["o_mem"] for i in range(N_CORES)], axis=0)
    c_wu = np.concatenate([r[i]["o_cwu"] for i in range(N_CORES)], axis=0)
    c_wlu = np.concatenate([r[i]["o_cwlu"] for i in range(N_CORES)], axis=0)
    c_wr = np.concatenate([r[i]["o_cwr"] for i in range(N_CORES)], axis=0)
    c_ww = np.concatenate([r[i]["o_cww"] for i in range(N_CORES)], axis=0)
    return read, memory, c_wu, c_wlu, c_wr, c_ww, h, c


# revision 29
# speedup vs baseline: 1.0772x; 1.0110x over previous
"""MANN LSTMCell step (scatter_memory) on 8 Trainium2 NeuronCores.

Sharding: the 32768-row memory axis is split 4096 rows/core (softmax over the
batch axis is per-mem-row, so it stays local; the memory write needs no
all-reduce in this decomposition).  Cross-core communication is one tiny
AllGather (per-batch local minima of the usage matrix, 1 KB) and one 128 KB
AllReduce (partial read vectors).  The LSTM controller is replicated on every
core.

The cosine-similarity matmul runs in fp32 (its result feeds an argmin whose
safety margin is ~4e-6); the read and write matmuls run in bf16 (their
outputs have orders-of-magnitude looser tolerances).
"""
import sys
import numpy as np

sys.path.insert(0, '/opt/trn_rl_repo')

MEM, UNITS, BATCH, IN_DIM = 32768, 128, 256, 512
N_CORES = 8
SHARD = MEM // N_CORES          # 4096 mem rows per core
T = SHARD // 128                # 32 tiles of 128 rows
CH = 4                          # tiles per DMA chunk (512 KB chunks)
NCHUNK = T // CH
USAGE_DECAY = 0.95

_state = {}


def _build():
    import concourse.bass as bass
    import concourse.bacc as bacc
    import concourse.mybir as mybir
    import concourse.tile as tile

    f32 = mybir.dt.float32
    bf16 = mybir.dt.bfloat16
    AF = mybir.ActivationFunctionType
    ALU = mybir.AluOpType

    nc = bacc.Bacc("TRN2", target_bir_lowering=False, debug=False,
                   num_devices=N_CORES)

    def din(name, shape):
        return nc.dram_tensor(name, shape, f32, kind="ExternalInput").ap()

    def dout(name, shape):
        return nc.dram_tensor(name, shape, f32, kind="ExternalOutput").ap()

    inputs_d = din("inputs", [BATCH, IN_DIM])
    r_d = din("r_tm1", [BATCH, UNITS])
    h_d = din("h_tm1", [BATCH, UNITS])
    c_d = din("c_tm1", [BATCH, UNITS])
    k_d = din("kern", [IN_DIM + UNITS, 4 * UNITS])
    rk_d = din("rec_kern", [UNITS, 4 * UNITS])
    b_d = din("bias", [1, 4 * UNITS])
    wg_d = din("wg", [1, 1])
    omw_d = din("omw", [1, 1])
    ident_d = din("ident", [128, 128])
    m_d = din("m_shard", [SHARD, UNITS])
    cwu0_d = din("c_wu_tm1", [SHARD, BATCH])
    cwlu0_d = din("c_wlu_tm1", [SHARD, BATCH])
    cwr0_d = din("c_wr_tm1", [SHARD, BATCH])

    o_cwr = dout("o_cwr", [SHARD, BATCH])
    o_cww = dout("o_cww", [SHARD, BATCH])
    o_cwu = dout("o_cwu", [SHARD, BATCH])
    o_cwlu = dout("o_cwlu", [SHARD, BATCH])
    o_mem = dout("o_mem", [SHARD, UNITS])
    o_read = dout("o_read", [BATCH, UNITS])
    o_h = dout("o_h", [BATCH, UNITS])
    o_c = dout("o_c", [BATCH, UNITS])

    with tile.TileContext(nc) as tc:
        with tc.tile_pool(name="const", bufs=1) as const, \
             tc.tile_pool(name="big", bufs=1) as big, \
             tc.tile_pool(name="wts", bufs=1) as wts, \
             tc.tile_pool(name="stream", bufs=3) as stream, \
             tc.tile_pool(name="sm", bufs=3) as sm, \
             tc.tile_pool(name="ps_read", bufs=1, space="PSUM") as ps_read, \
             tc.tile_pool(name="ps_mc", bufs=2, space="PSUM") as ps_mc, \
             tc.tile_pool(name="ps_tp", bufs=2, space="PSUM") as ps_tp, \
             tc.tile_pool(name="ps_w", bufs=2, space="PSUM") as ps_w, \
             tc.tile_pool(name="dram", bufs=1, space="DRAM") as dram:

            # ---------------- constants ----------------
            id_t = const.tile([128, 128], f32)
            nc.sync.dma_start(id_t[:], ident_d[:])
            id16 = const.tile([128, 128], bf16)
            nc.vector.tensor_copy(id16[:], id_t[:])
            id95 = const.tile([128, 128], f32)
            nc.vector.tensor_scalar(id95[:], id_t[:], USAGE_DECAY, None, ALU.mult)

            wg_raw = const.tile([1, 1], f32)
            omw_raw = const.tile([1, 1], f32)
            nc.sync.dma_start(wg_raw[:], wg_d[:])
            nc.sync.dma_start(omw_raw[:], omw_d[:])
            wg_b = const.tile([128, 1], f32)
            omw_b = const.tile([128, 1], f32)
            nc.gpsimd.partition_broadcast(wg_b[:], wg_raw[:])
            nc.gpsimd.partition_broadcast(omw_b[:], omw_raw[:])

            bias_row = const.tile([1, 512], f32)
            nc.sync.dma_start(bias_row[:], b_d[:])
            bias_b = const.tile([128, 512], f32)
            nc.gpsimd.partition_broadcast(bias_b[:], bias_row[:])

            # ---------------- LSTM controller (replicated) ----------------
            inp_sb = []
            r_sb = []
            h_sb = []
            c_sb = []
            for bh in range(2):
                t_in = wts.tile([128, 512], f32, name=f"inp{bh}")
                nc.sync.dma_start(t_in[:], inputs_d[bh * 128:(bh + 1) * 128, :])
                inp_sb.append(t_in)
                t_r = wts.tile([128, 128], f32, name=f"r{bh}")
                nc.sync.dma_start(t_r[:], r_d[bh * 128:(bh + 1) * 128, :])
                r_sb.append(t_r)
                t_h = wts.tile([128, 128], f32, name=f"h{bh}")
                nc.sync.dma_start(t_h[:], h_d[bh * 128:(bh + 1) * 128, :])
                h_sb.append(t_h)
                t_c = wts.tile([128, 128], f32, name=f"c{bh}")
                nc.sync.dma_start(t_c[:], c_d[bh * 128:(bh + 1) * 128, :])
                c_sb.append(t_c)

            k_sb = []
            for kb in range(5):
                t_k = wts.tile([128, 512], f32, name=f"k{kb}")
                nc.sync.dma_start(t_k[:], k_d[kb * 128:(kb + 1) * 128, :])
                k_sb.append(t_k)
            rk_sb = wts.tile([128, 512], f32)
            nc.sync.dma_start(rk_sb[:], rk_d[:])

            # transposed concat([inputs, r], -1): ciT[kb] is [128k, 256b]
            ciT = [wts.tile([128, 256], f32, name=f"ciT{kb}") for kb in range(5)]
            hT = wts.tile([128, 256], f32)
            for bh in range(2):
                for fb in range(4):
                    pt = ps_tp.tile([128, 128], f32, tag="tp")
                    nc.tensor.transpose(pt[:], inp_sb[bh][:, fb * 128:(fb + 1) * 128], id_t[:])
                    nc.scalar.activation(ciT[fb][:, bh * 128:(bh + 1) * 128], pt[:], AF.Copy)
                pt = ps_tp.tile([128, 128], f32, tag="tp")
                nc.tensor.transpose(pt[:], r_sb[bh][:], id_t[:])
                nc.scalar.activation(ciT[4][:, bh * 128:(bh + 1) * 128], pt[:], AF.Copy)
                pt = ps_tp.tile([128, 128], f32, tag="tp")
                nc.tensor.transpose(pt[:], h_sb[bh][:], id_t[:])
                nc.scalar.activation(hT[:, bh * 128:(bh + 1) * 128], pt[:], AF.Copy)

            h_new = []
            h16 = []
            f32r = mybir.dt.float32r
            nkey = const.tile([128, 256], f32r)
            for bh in range(2):
                zp = ps_mc.tile([128, 512], f32, tag="mc")
                for kb in range(5):
                    nc.tensor.matmul(zp[:], lhsT=ciT[kb][:, bh * 128:(bh + 1) * 128],
                                     rhs=k_sb[kb][:], start=(kb == 0), stop=False)
                nc.tensor.matmul(zp[:], lhsT=hT[:, bh * 128:(bh + 1) * 128],
                                 rhs=rk_sb[:], start=False, stop=True)
                z_sb = wts.tile([128, 512], f32, name=f"z{bh}", tag=f"inp{bh}")
                nc.vector.tensor_tensor(z_sb[:], zp[:], bias_b[:], ALU.add)

                ig = sm.tile([128, 128], f32, bufs=1)
                fg = sm.tile([128, 128], f32, bufs=1)
                gg = sm.tile([128, 128], f32, bufs=1)
                og = sm.tile([128, 128], f32, bufs=1)
                nc.scalar.activation(ig[:], z_sb[:, 0:128], AF.Sigmoid)
                nc.scalar.activation(fg[:], z_sb[:, 128:256], AF.Sigmoid)
                nc.scalar.activation(gg[:], z_sb[:, 256:384], AF.Tanh)
                nc.scalar.activation(og[:], z_sb[:, 384:512], AF.Sigmoid)
                t_ig = sm.tile([128, 128], f32, bufs=1)
                nc.vector.tensor_tensor(t_ig[:], ig[:], gg[:], ALU.mult)
                t_fc = sm.tile([128, 128], f32, bufs=1)
                nc.vector.tensor_tensor(t_fc[:], fg[:], c_sb[bh][:], ALU.mult)
                cn = const.tile([128, 128], f32, name=f"cnew{bh}")
                nc.vector.tensor_tensor(cn[:], t_fc[:], t_ig[:], ALU.add)
                tc_ = sm.tile([128, 128], f32, bufs=1)
                nc.scalar.activation(tc_[:], cn[:], AF.Tanh)
                hn = const.tile([128, 128], f32, name=f"hnew{bh}")
                nc.vector.tensor_tensor(hn[:], og[:], tc_[:], ALU.mult)
                h_new.append(hn)
                nc.sync.dma_start(o_h[bh * 128:(bh + 1) * 128, :], hn[:])
                nc.sync.dma_start(o_c[bh * 128:(bh + 1) * 128, :], cn[:])
                hb = const.tile([128, 128], bf16, name=f"h16_{bh}")
                nc.vector.tensor_copy(hb[:], hn[:])
                h16.append(hb)

                # l2-normalize rows of h (= columns of key_list)
                sq_s = sm.tile([128, 128], f32, bufs=1)
                ss = sm.tile([128, 1], f32)
                nc.scalar.activation(sq_s[:], hn[:], AF.Square, accum_out=ss[:])
                ssm = sm.tile([128, 1], f32)
                nc.vector.tensor_scalar(ssm[:], ss[:], 1e-12, None, ALU.max)
                sq2 = sm.tile([128, 1], f32)
                nc.scalar.activation(sq2[:], ssm[:], AF.Sqrt)
                rr = sm.tile([128, 1], f32)
                nc.vector.reciprocal(rr[:], sq2[:])
                nh = sm.tile([128, 128], f32, bufs=1)
                nc.vector.tensor_scalar(nh[:], hn[:], rr[:], None, ALU.mult)
                pt = ps_tp.tile([128, 128], f32, tag="tp")
                nc.tensor.transpose(pt[:], nh[:], id_t[:])
                nc.scalar.activation(nkey[:, bh * 128:(bh + 1) * 128], pt[:], AF.Copy)

            # ---------------- persistent big buffers ----------------
            m_all = big.tile([128, T * 128], f32)
            cwu_all = big.tile([128, T * 256], f32)
            w_all = big.tile([128, T * 128], f32)
            min_run = const.tile([128, 256], f32)

            read_a = ps_read.tile([128, 128], f32)
            read_b = ps_read.tile([128, 128], f32)

            # ---------------- phase A: main streaming loop ----------------
            # tapered: the last chunks shrink so the local min (and with it
            # the AllGather) launches as early as possible
            chunk_plan = []
            base = 0
            for ch in (4, 4, 4, 4, 4, 4, 4, 4):
                chunk_plan.append((base, ch))
                base += ch
            assert base == T
            for ci, (cb, ch) in enumerate(chunk_plan):
                rows = slice(cb * 128, (cb + ch) * 128)
                csl = slice(cb * 256, (cb + ch) * 256)
                msl = slice(cb * 128, (cb + ch) * 128)
                cwu_in = stream.tile([128, CH * 256], f32, tag="cwu_in")
                cwr_in = stream.tile([128, CH * 256], f32, tag="cwr_in")
                cwlu_in = stream.tile([128, CH * 256], f32, tag="cwlu_in")
                nc.sync.dma_start(
                    cwu_in[:, 0:ch * 256].rearrange("p (t b) -> p t b", b=BATCH),
                    cwu0_d[rows, :].rearrange("(t p) b -> p t b", p=128))
                nc.sync.dma_start(
                    cwr_in[:, 0:ch * 256].rearrange("p (t b) -> p t b", b=BATCH),
                    cwr0_d[rows, :].rearrange("(t p) b -> p t b", p=128))
                nc.sync.dma_start(
                    cwlu_in[:, 0:ch * 256].rearrange("p (t b) -> p t b", b=BATCH),
                    cwlu0_d[rows, :].rearrange("(t p) b -> p t b", p=128))
                nc.sync.dma_start(
                    m_all[:, msl].rearrange("p (t u) -> p t u", u=UNITS),
                    m_d[rows, :].rearrange("(t p) u -> p t u", p=128))

                cwr_out = stream.tile([128, CH * 256], f32, tag="cwr_out")
                cww_out = stream.tile([128, CH * 256], f32, tag="cww_out")

                # c_ww chunk: affine into cww_out, then += c_wlu_tm1 (in place)
                nc.vector.tensor_scalar(cww_out[:, 0:ch * 256], cwr_in[:, 0:ch * 256], wg_b[:], omw_b[:],
                                        ALU.mult, ALU.add)
                nc.vector.tensor_tensor(cww_out[:, 0:ch * 256], cww_out[:, 0:ch * 256], cwlu_in[:, 0:ch * 256], ALU.add)
                cww16 = stream.tile([128, CH * 256], bf16, tag="cww16", bufs=1)
                nc.vector.tensor_copy(cww16[:, 0:ch * 256], cww_out[:, 0:ch * 256])

                # batched row-norm stats for this chunk
                ssm_c = sm.tile([128, CH], f32, tag="ssm_c")
                sqr_c = sm.tile([128, CH], f32, tag="sqr_c")
                rr_c = sm.tile([128, CH], f32, tag="rr_c")

                # row-norm sums of squares for the chunk, on DVE
                sqc = sm.tile([128, CH * 128], f32, tag="sqc", bufs=2)
                nc.vector.tensor_tensor(sqc[:, 0:ch * 128], m_all[:, msl],
                                        m_all[:, msl], ALU.mult)
                nc.vector.tensor_reduce(
                    ssm_c[:, 0:ch],
                    sqc[:, 0:ch * 128].rearrange("p (t u) -> p t u", u=UNITS),
                    mybir.AxisListType.X, ALU.add)

                # pass 1: write matmuls
                for t in range(ch):
                    tt = cb + t
                    m_t = m_all[:, tt * 128:(tt + 1) * 128]

                    wt16a = sm.tile([128, 128], bf16, tag="wt16a", bufs=2)
                    wt16b = sm.tile([128, 128], bf16, tag="wt16b", bufs=2)
                    pta = ps_tp.tile([128, 128], bf16, tag="tp")
                    nc.tensor.transpose(pta[:], cww16[:, t * 256:t * 256 + 128], id16[:])
                    nc.vector.tensor_copy(wt16a[:], pta[:])
                    ptb = ps_tp.tile([128, 128], bf16, tag="tp")
                    nc.tensor.transpose(ptb[:], cww16[:, t * 256 + 128:(t + 1) * 256], id16[:])
                    nc.vector.tensor_copy(wt16b[:], ptb[:])
                    w_ps = ps_w.tile([128, 128], f32, tag="wps")
                    nc.tensor.matmul(w_ps[:], lhsT=wt16a[:], rhs=h16[0][:],
                                     start=True, stop=False)
                    nc.tensor.matmul(w_ps[:], lhsT=wt16b[:], rhs=h16[1][:],
                                     start=False, stop=True)
                    nc.vector.tensor_copy(w_all[:, tt * 128:(tt + 1) * 128], w_ps[:])

                # sqrt + reciprocal of the row norms, batched per chunk
                nc.vector.tensor_scalar(ssm_c[:, 0:ch], ssm_c[:, 0:ch], 1e-12, None, ALU.max)
                nc.scalar.activation(sqr_c[:, 0:ch], ssm_c[:, 0:ch], AF.Sqrt)
                nc.vector.reciprocal(rr_c[:, 0:ch], sqr_c[:, 0:ch])

                # pass 2: transpose m, cosine matmul, softmax
                for t in range(ch):
                    tt = cb + t
                    m_t = m_all[:, tt * 128:(tt + 1) * 128]
                    bsl = slice(t * 256, (t + 1) * 256)

                    ptm = ps_tp.tile([128, 128], f32, tag="tp")
                    nc.tensor.transpose(ptm[:], m_t, id_t[:])
                    mT = sm.tile([128, 128], f32r, tag="mT")
                    nc.scalar.activation(mT[:], ptm[:], AF.Copy)

                    mc = ps_mc.tile([128, 256], f32, tag="mc")
                    nc.tensor.matmul(mc[:], lhsT=mT[:], rhs=nkey[:],
                                     start=True, stop=True)

                    et = sm.tile([128, 256], f32, tag="exp", bufs=2)
                    se = sm.tile([128, 1], f32, tag="se")
                    nc.scalar.activation(et[:], mc[:], AF.Exp,
                                         scale=rr_c[:, t:t + 1], accum_out=se[:])
                    rse = sm.tile([128, 1], f32, tag="rse")
                    nc.vector.reciprocal(rse[:], se[:])
                    nc.vector.tensor_scalar(cwr_out[:, bsl], et[:], rse[:], None,
                                            ALU.mult)

                # read matmuls (fp32)
                for t in range(ch):
                    tt = cb + t
                    nc.tensor.matmul(read_a[:], lhsT=cwr_out[:, t * 256:t * 256 + 128],
                                     rhs=m_all[:, tt * 128:(tt + 1) * 128],
                                     start=(tt == 0), stop=(tt == T - 1))
                    nc.tensor.matmul(read_b[:], lhsT=cwr_out[:, t * 256 + 128:(t + 1) * 256],
                                     rhs=m_all[:, tt * 128:(tt + 1) * 128],
                                     start=(tt == 0), stop=(tt == T - 1))

                # c_wu chunk: 0.95*c_wu_tm1 + c_wr + c_ww  (in place in cwu_in)
                nc.vector.tensor_scalar(cwu_in[:, 0:ch * 256], cwu_in[:, 0:ch * 256],
                                        USAGE_DECAY, None, ALU.mult)
                nc.vector.tensor_tensor(cwu_in[:, 0:ch * 256], cwu_in[:, 0:ch * 256],
                                        cwr_out[:, 0:ch * 256], ALU.add)
                nc.vector.tensor_tensor(cwu_all[:, csl], cwu_in[:, 0:ch * 256],
                                        cww_out[:, 0:ch * 256], ALU.add)

                # running per-batch minimum: one strided reduce over the
                # chunk's tile axis, then fold into the running min
                cwuc_v = cwu_all[:, csl].rearrange("p (t b) -> p b t", b=BATCH)
                mtree = sm.tile([128, 256], f32, tag="mtree", bufs=1)
                nc.vector.tensor_reduce(mtree[:], cwuc_v, mybir.AxisListType.X,
                                        ALU.min)
                if ci == 0:
                    nc.vector.tensor_copy(min_run[:], mtree[:])
                else:
                    nc.vector.tensor_tensor(min_run[:], min_run[:], mtree[:],
                                            ALU.min)

                # chunk outputs
                nc.sync.dma_start(
                    o_cwr[rows, :].rearrange("(t p) b -> p t b", p=128),
                    cwr_out[:, 0:ch * 256].rearrange("p (t b) -> p t b", b=BATCH))
                nc.sync.dma_start(
                    o_cww[rows, :].rearrange("(t p) b -> p t b", p=128),
                    cww_out[:, 0:ch * 256].rearrange("p (t b) -> p t b", b=BATCH))
            # ---------------- global min via AllGather ----------------
            negmin = const.tile([128, 256], f32)
            nc.vector.tensor_scalar(negmin[:], min_run[:], -1.0, None, ALU.mult)
            negred = const.tile([128, 256], f32)
            nc.gpsimd.partition_all_reduce(negred[:], negmin[:], channels=128,
                                           reduce_op=_reduce_max())
            mn_in = dram.tile([1, 256], f32)
            mn_out = dram.tile([N_CORES, 256], f32, addr_space="Shared")
            nc.sync.dma_start(mn_in[:], negred[0:1, :])
            cc_ag = nc.gpsimd.collective_compute(
                "AllGather", mybir.AluOpType.bypass,
                replica_groups=[list(range(N_CORES))],
                ins=[mn_in[:].opt()], outs=[mn_out[:].opt()])
            # c_wu output writeback deferred to here: it reads the persistent
            # cwu_all buffer and fills the AllGather skew window with DMA work
            for ci in range(NCHUNK):
                rows = slice(ci * CH * 128, (ci + 1) * CH * 128)
                csl = slice(ci * CH * 256, (ci + 1) * CH * 256)
                nc.sync.dma_start(
                    o_cwu[rows, :].rearrange("(t p) b -> p t b", p=128),
                    cwu_all[:, csl].rearrange("p (t b) -> p t b", b=BATCH))
            negall = const.tile([N_CORES, 256], f32)
            nc.sync.dma_start(negall[:], mn_out[:])
            negall_r = const.tile([N_CORES, 256], f32)
            nc.gpsimd.partition_all_reduce(negall_r[:], negall[:], channels=N_CORES,
                                           reduce_op=_reduce_max())
            gmin_row = const.tile([1, 256], f32)
            nc.vector.tensor_scalar(gmin_row[:], negall_r[0:1, :], -1.0, None,
                                    ALU.mult)
            gmin_b = const.tile([128, 256], f32)
            nc.gpsimd.partition_broadcast(gmin_b[:], gmin_row[:])
            gmin_rep = gmin_b[:].rearrange("p (x b) -> p x b", x=1) \
                                .broadcast_to([128, CH, 256])

            # ---------------- phase C: c_wlu + memory write ----------------
            for ci in range(NCHUNK):
                rows = slice(ci * CH * 128, (ci + 1) * CH * 128)
                csl = slice(ci * CH * 256, (ci + 1) * CH * 256)
                msl = slice(ci * CH * 128, (ci + 1) * CH * 128)
                # reuse phase-A stream slots (those tags are dead by now)
                cwlu_out = stream.tile([128, CH * 256], f32, tag="cwu_in")
                memc = stream.tile([128, CH * 128], f32, tag="cwr_out")
                nc.vector.tensor_tensor(
                    cwlu_out[:].rearrange("p (t b) -> p t b", b=BATCH),
                    cwu_all[:, csl].rearrange("p (t b) -> p t b", b=BATCH),
                    gmin_rep, ALU.is_le)
                cnt_c = sm.tile([128, CH], f32, tag="cnt_c")
                nc.vector.tensor_reduce(
                    cnt_c[:], cwlu_out[:].rearrange("p (t b) -> p t b", b=BATCH),
                    mybir.AxisListType.X, ALU.add)
                scl_c = sm.tile([128, CH], f32, tag="scl_c")
                nc.vector.tensor_scalar(scl_c[:], cnt_c[:], -1.0, float(BATCH),
                                        ALU.mult, ALU.add)
                for t in range(CH):
                    tt = ci * CH + t
                    nc.scalar.activation(memc[:, t * 128:(t + 1) * 128],
                                         m_all[:, tt * 128:(tt + 1) * 128],
                                         AF.Identity, scale=scl_c[:, t:t + 1])
                nc.vector.tensor_tensor(memc[:], memc[:], w_all[:, msl], ALU.add)
                nc.sync.dma_start(
                    o_cwlu[rows, :].rearrange("(t p) b -> p t b", p=128),
                    cwlu_out[:].rearrange("p (t b) -> p t b", b=BATCH))
                nc.sync.dma_start(
                    o_mem[rows, :].rearrange("(t p) u -> p t u", p=128),
                    memc[:].rearrange("p (t u) -> p t u", u=UNITS))

            # ---------------- read partial all-reduce (off critical path) ----
            read_sb = const.tile([128, 256], f32)
            nc.vector.tensor_copy(read_sb[:, 0:128], read_a[:])
            nc.vector.tensor_copy(read_sb[:, 128:256], read_b[:])
            rd_in = dram.tile([BATCH, UNITS], f32)
            rd_out = dram.tile([BATCH, UNITS], f32, addr_space="Shared")
            nc.sync.dma_start(rd_in[0:128, :], read_sb[:, 0:128])
            nc.sync.dma_start(rd_in[128:256, :], read_sb[:, 128:256])
            cc_ar = nc.gpsimd.collective_compute(
                "AllReduce", mybir.AluOpType.add,
                replica_groups=[list(range(N_CORES))],
                ins=[rd_in[:].opt()], outs=[rd_out[:].opt()])
            from concourse.tile_rust import add_dep_helper
            add_dep_helper(cc_ar.ins, cc_ag.ins, sync=True,
                           reason="AllGather feeds the critical path; run it first")
            nc.sync.dma_start(o_read[:], rd_out[:])

    nc.compile()
    return nc


def _reduce_max():
    from concourse import bass_isa
    return bass_isa.ReduceOp.max


def _ensure_built():
    if "nc" not in _state:
        _state["nc"] = _build()
    return _state["nc"]


def kernel(**inputs):
    from concourse import bass_utils

    nc = _ensure_built()

    inp = {k: np.asarray(v, dtype=np.float32) for k, v in inputs.items()}
    wg = 1.0 / (1.0 + np.exp(-inp["write_gate"].astype(np.float64)))
    wg32 = wg.astype(np.float32).reshape(1, 1)
    omw32 = (1.0 - wg32).astype(np.float32)

    shared = {
        "inputs": inp["inputs"],
        "r_tm1": inp["r_tm1"],
        "h_tm1": inp["h_tm1"],
        "c_tm1": inp["c_tm1"],
        "kern": inp["kernel"],
        "rec_kern": inp["rec_kernel"],
        "bias": inp["bias"].reshape(1, 4 * UNITS),
        "wg": wg32,
        "omw": omw32,
        "ident": np.eye(128, dtype=np.float32),
    }
    in_maps = []
    for c in range(N_CORES):
        rows = slice(c * SHARD, (c + 1) * SHARD)
        m = dict(shared)
        m["m_shard"] = np.ascontiguousarray(inp["m_tm1"][rows])
        m["c_wu_tm1"] = np.ascontiguousarray(inp["c_wu_tm1"][rows])
        m["c_wlu_tm1"] = np.ascontiguousarray(inp["c_wlu_tm1"][rows])
        m["c_wr_tm1"] = np.ascontiguousarray(inp["c_wr_tm1"][rows])
        in_maps.append(m)

    res = bass_utils.run_bass_kernel_spmd(
        nc, in_maps, core_ids=list(range(N_CORES)),
        trace=bool(_state.get("trace", False)))
    _state["last_result"] = res
    r = res.results

    read = r[0]["o_read"]
    h = r[0]["o_h"]
    c = r[0]["o_c"]
    memory = np.concatenate([r[i]["o_mem"] for i in range(N_CORES)], axis=0)
    c_wu = np.concatenate([r[i]["o_cwu"] for i in range(N_CORES)], axis=0)
    c_wlu = np.concatenate([r[i]["o_cwlu"] for i in range(N_CORES)], axis=0)
    c_wr = np.concatenate([r[i]["o_cwr"] for i in range(N_CORES)], axis=0)
    c_ww = np.concatenate([r[i]["o_cww"] for i in range(N_CORES)], axis=0)
    return read, memory, c_wu, c_wlu, c_wr, c_ww, h, c


# revision 30
# speedup vs baseline: 1.1508x; 1.0683x over previous
"""MANN LSTMCell step (scatter_memory) on 8 Trainium2 NeuronCores.

Sharding: the 32768-row memory axis is split 4096 rows/core (softmax over the
batch axis is per-mem-row, so it stays local; the memory write needs no
all-reduce in this decomposition).  Cross-core communication is one tiny
AllGather (per-batch local minima of the usage matrix, 1 KB) and one 128 KB
AllReduce (partial read vectors).  The LSTM controller is replicated on every
core.

The cosine-similarity matmul runs in fp32 (its result feeds an argmin whose
safety margin is ~4e-6); the read and write matmuls run in bf16 (their
outputs have orders-of-magnitude looser tolerances).
"""
import sys
import numpy as np

sys.path.insert(0, '/opt/trn_rl_repo')

MEM, UNITS, BATCH, IN_DIM = 32768, 128, 256, 512
N_CORES = 8
SHARD = MEM // N_CORES          # 4096 mem rows per core
T = SHARD // 128                # 32 tiles of 128 rows
CH = 4                          # tiles per DMA chunk (512 KB chunks)
NCHUNK = T // CH
USAGE_DECAY = 0.95

_state = {}


def _build():
    import concourse.bass as bass
    import concourse.bacc as bacc
    import concourse.mybir as mybir
    import concourse.tile as tile

    f32 = mybir.dt.float32
    bf16 = mybir.dt.bfloat16
    AF = mybir.ActivationFunctionType
    ALU = mybir.AluOpType

    nc = bacc.Bacc("TRN2", target_bir_lowering=False, debug=False,
                   num_devices=N_CORES)

    def din(name, shape):
        return nc.dram_tensor(name, shape, f32, kind="ExternalInput").ap()

    def dout(name, shape):
        return nc.dram_tensor(name, shape, f32, kind="ExternalOutput").ap()

    inputs_d = din("inputs", [BATCH, IN_DIM])
    r_d = din("r_tm1", [BATCH, UNITS])
    h_d = din("h_tm1", [BATCH, UNITS])
    c_d = din("c_tm1", [BATCH, UNITS])
    k_d = din("kern", [IN_DIM + UNITS, 4 * UNITS])
    rk_d = din("rec_kern", [UNITS, 4 * UNITS])
    b_d = din("bias", [1, 4 * UNITS])
    wg_d = din("wg", [1, 1])
    omw_d = din("omw", [1, 1])
    ident_d = din("ident", [128, 128])
    m_d = din("m_shard", [SHARD, UNITS])
    cwu0_d = din("c_wu_tm1", [SHARD, BATCH])
    cwlu0_d = din("c_wlu_tm1", [SHARD, BATCH])
    cwr0_d = din("c_wr_tm1", [SHARD, BATCH])

    o_cwr = dout("o_cwr", [SHARD, BATCH])
    o_cww = dout("o_cww", [SHARD, BATCH])
    o_cwu = dout("o_cwu", [SHARD, BATCH])
    o_cwlu = dout("o_cwlu", [SHARD, BATCH])
    o_mem = dout("o_mem", [SHARD, UNITS])
    o_read = dout("o_read", [BATCH, UNITS])
    o_h = dout("o_h", [BATCH, UNITS])
    o_c = dout("o_c", [BATCH, UNITS])

    with tile.TileContext(nc) as tc:
        with tc.tile_pool(name="const", bufs=1) as const, \
             tc.tile_pool(name="big", bufs=1) as big, \
             tc.tile_pool(name="wts", bufs=1) as wts, \
             tc.tile_pool(name="stream", bufs=3) as stream, \
             tc.tile_pool(name="sm", bufs=3) as sm, \
             tc.tile_pool(name="ps_read", bufs=1, space="PSUM") as ps_read, \
             tc.tile_pool(name="ps_mc", bufs=2, space="PSUM") as ps_mc, \
             tc.tile_pool(name="ps_tp", bufs=2, space="PSUM") as ps_tp, \
             tc.tile_pool(name="ps_w", bufs=2, space="PSUM") as ps_w, \
             tc.tile_pool(name="dram", bufs=1, space="DRAM") as dram:

            # ---------------- constants ----------------
            id_t = const.tile([128, 128], f32)
            nc.sync.dma_start(id_t[:], ident_d[:])
            id16 = const.tile([128, 128], bf16)
            nc.vector.tensor_copy(id16[:], id_t[:])
            id95 = const.tile([128, 128], f32)
            nc.vector.tensor_scalar(id95[:], id_t[:], USAGE_DECAY, None, ALU.mult)

            wg_raw = const.tile([1, 1], f32)
            omw_raw = const.tile([1, 1], f32)
            nc.sync.dma_start(wg_raw[:], wg_d[:])
            nc.sync.dma_start(omw_raw[:], omw_d[:])
            wg_b = const.tile([128, 1], f32)
            omw_b = const.tile([128, 1], f32)
            nc.gpsimd.partition_broadcast(wg_b[:], wg_raw[:])
            nc.gpsimd.partition_broadcast(omw_b[:], omw_raw[:])

            bias_row = const.tile([1, 512], f32)
            nc.sync.dma_start(bias_row[:], b_d[:])
            bias_b = const.tile([128, 512], f32)
            nc.gpsimd.partition_broadcast(bias_b[:], bias_row[:])

            # ---------------- LSTM controller (replicated) ----------------
            inp_sb = []
            r_sb = []
            h_sb = []
            c_sb = []
            for bh in range(2):
                t_in = wts.tile([128, 512], f32, name=f"inp{bh}")
                nc.sync.dma_start(t_in[:], inputs_d[bh * 128:(bh + 1) * 128, :])
                inp_sb.append(t_in)
                t_r = wts.tile([128, 128], f32, name=f"r{bh}")
                nc.sync.dma_start(t_r[:], r_d[bh * 128:(bh + 1) * 128, :])
                r_sb.append(t_r)
                t_h = wts.tile([128, 128], f32, name=f"h{bh}")
                nc.sync.dma_start(t_h[:], h_d[bh * 128:(bh + 1) * 128, :])
                h_sb.append(t_h)
                t_c = wts.tile([128, 128], f32, name=f"c{bh}")
                nc.sync.dma_start(t_c[:], c_d[bh * 128:(bh + 1) * 128, :])
                c_sb.append(t_c)

            k_sb = []
            for kb in range(5):
                t_k = wts.tile([128, 512], f32, name=f"k{kb}")
                nc.sync.dma_start(t_k[:], k_d[kb * 128:(kb + 1) * 128, :])
                k_sb.append(t_k)
            rk_sb = wts.tile([128, 512], f32)
            nc.sync.dma_start(rk_sb[:], rk_d[:])

            # transposed concat([inputs, r], -1): ciT[kb] is [128k, 256b]
            ciT = [wts.tile([128, 256], f32, name=f"ciT{kb}") for kb in range(5)]
            hT = wts.tile([128, 256], f32)
            for bh in range(2):
                for fb in range(4):
                    pt = ps_tp.tile([128, 128], f32, tag="tp")
                    nc.tensor.transpose(pt[:], inp_sb[bh][:, fb * 128:(fb + 1) * 128], id_t[:])
                    nc.scalar.activation(ciT[fb][:, bh * 128:(bh + 1) * 128], pt[:], AF.Copy)
                pt = ps_tp.tile([128, 128], f32, tag="tp")
                nc.tensor.transpose(pt[:], r_sb[bh][:], id_t[:])
                nc.scalar.activation(ciT[4][:, bh * 128:(bh + 1) * 128], pt[:], AF.Copy)
                pt = ps_tp.tile([128, 128], f32, tag="tp")
                nc.tensor.transpose(pt[:], h_sb[bh][:], id_t[:])
                nc.scalar.activation(hT[:, bh * 128:(bh + 1) * 128], pt[:], AF.Copy)

            h_new = []
            h16 = []
            f32r = mybir.dt.float32r
            nkey = const.tile([128, 256], f32r)
            for bh in range(2):
                zp = ps_mc.tile([128, 512], f32, tag="mc")
                for kb in range(5):
                    nc.tensor.matmul(zp[:], lhsT=ciT[kb][:, bh * 128:(bh + 1) * 128],
                                     rhs=k_sb[kb][:], start=(kb == 0), stop=False)
                nc.tensor.matmul(zp[:], lhsT=hT[:, bh * 128:(bh + 1) * 128],
                                 rhs=rk_sb[:], start=False, stop=True)
                z_sb = wts.tile([128, 512], f32, name=f"z{bh}", tag=f"inp{bh}")
                nc.vector.tensor_tensor(z_sb[:], zp[:], bias_b[:], ALU.add)

                ig = sm.tile([128, 128], f32, bufs=1)
                fg = sm.tile([128, 128], f32, bufs=1)
                gg = sm.tile([128, 128], f32, bufs=1)
                og = sm.tile([128, 128], f32, bufs=1)
                nc.scalar.activation(ig[:], z_sb[:, 0:128], AF.Sigmoid)
                nc.scalar.activation(fg[:], z_sb[:, 128:256], AF.Sigmoid)
                nc.scalar.activation(gg[:], z_sb[:, 256:384], AF.Tanh)
                nc.scalar.activation(og[:], z_sb[:, 384:512], AF.Sigmoid)
                t_ig = sm.tile([128, 128], f32, bufs=1)
                nc.vector.tensor_tensor(t_ig[:], ig[:], gg[:], ALU.mult)
                t_fc = sm.tile([128, 128], f32, bufs=1)
                nc.vector.tensor_tensor(t_fc[:], fg[:], c_sb[bh][:], ALU.mult)
                cn = const.tile([128, 128], f32, name=f"cnew{bh}")
                nc.vector.tensor_tensor(cn[:], t_fc[:], t_ig[:], ALU.add)
                tc_ = sm.tile([128, 128], f32, bufs=1)
                nc.scalar.activation(tc_[:], cn[:], AF.Tanh)
                hn = const.tile([128, 128], f32, name=f"hnew{bh}")
                nc.vector.tensor_tensor(hn[:], og[:], tc_[:], ALU.mult)
                h_new.append(hn)
                nc.sync.dma_start(o_h[bh * 128:(bh + 1) * 128, :], hn[:])
                nc.sync.dma_start(o_c[bh * 128:(bh + 1) * 128, :], cn[:])
                hb = const.tile([128, 128], bf16, name=f"h16_{bh}")
                nc.vector.tensor_copy(hb[:], hn[:])
                h16.append(hb)

                # l2-normalize rows of h (= columns of key_list)
                sq_s = sm.tile([128, 128], f32, bufs=1)
                ss = sm.tile([128, 1], f32)
                nc.scalar.activation(sq_s[:], hn[:], AF.Square, accum_out=ss[:])
                ssm = sm.tile([128, 1], f32)
                nc.vector.tensor_scalar(ssm[:], ss[:], 1e-12, None, ALU.max)
                sq2 = sm.tile([128, 1], f32)
                nc.scalar.activation(sq2[:], ssm[:], AF.Sqrt)
                rr = sm.tile([128, 1], f32)
                nc.vector.reciprocal(rr[:], sq2[:])
                nh = sm.tile([128, 128], f32, bufs=1)
                nc.vector.tensor_scalar(nh[:], hn[:], rr[:], None, ALU.mult)
                pt = ps_tp.tile([128, 128], f32, tag="tp")
                nc.tensor.transpose(pt[:], nh[:], id_t[:])
                nc.scalar.activation(nkey[:, bh * 128:(bh + 1) * 128], pt[:], AF.Copy)

            # ---------------- persistent big buffers ----------------
            m_all = big.tile([128, T * 128], f32)
            cwu_all = big.tile([128, T * 256], f32)
            w_all = big.tile([128, T * 128], f32)
            min_run = const.tile([128, 256], f32)

            read_a = ps_read.tile([128, 128], f32)
            read_b = ps_read.tile([128, 128], f32)

            # ---------------- phase A: main streaming loop ----------------
            # tapered: the last chunks shrink so the local min (and with it
            # the AllGather) launches as early as possible
            chunk_plan = []
            base = 0
            for ch in (4, 4, 4, 4, 4, 4, 4, 2, 2):
                chunk_plan.append((base, ch))
                base += ch
            assert base == T
            for ci, (cb, ch) in enumerate(chunk_plan):
                rows = slice(cb * 128, (cb + ch) * 128)
                csl = slice(cb * 256, (cb + ch) * 256)
                msl = slice(cb * 128, (cb + ch) * 128)
                cwu_in = stream.tile([128, CH * 256], f32, tag="cwu_in")
                cwr_in = stream.tile([128, CH * 256], f32, tag="cwr_in")
                cwlu_in = stream.tile([128, CH * 256], f32, tag="cwlu_in")
                nc.sync.dma_start(
                    cwu_in[:, 0:ch * 256].rearrange("p (t b) -> p t b", b=BATCH),
                    cwu0_d[rows, :].rearrange("(t p) b -> p t b", p=128))
                nc.sync.dma_start(
                    cwr_in[:, 0:ch * 256].rearrange("p (t b) -> p t b", b=BATCH),
                    cwr0_d[rows, :].rearrange("(t p) b -> p t b", p=128))
                nc.sync.dma_start(
                    cwlu_in[:, 0:ch * 256].rearrange("p (t b) -> p t b", b=BATCH),
                    cwlu0_d[rows, :].rearrange("(t p) b -> p t b", p=128))
                nc.sync.dma_start(
                    m_all[:, msl].rearrange("p (t u) -> p t u", u=UNITS),
                    m_d[rows, :].rearrange("(t p) u -> p t u", p=128))

                cwr_out = stream.tile([128, CH * 256], f32, tag="cwr_out")
                cww_out = stream.tile([128, CH * 256], f32, tag="cww_out")

                # c_ww chunk: affine into cww_out, then += c_wlu_tm1 (in place)
                nc.vector.tensor_scalar(cww_out[:, 0:ch * 256], cwr_in[:, 0:ch * 256], wg_b[:], omw_b[:],
                                        ALU.mult, ALU.add)
                nc.vector.tensor_tensor(cww_out[:, 0:ch * 256], cww_out[:, 0:ch * 256], cwlu_in[:, 0:ch * 256], ALU.add)
                cww16 = stream.tile([128, CH * 256], bf16, tag="cww16", bufs=1)
                nc.vector.tensor_copy(cww16[:, 0:ch * 256], cww_out[:, 0:ch * 256])

                # batched row-norm stats for this chunk
                ssm_c = sm.tile([128, CH], f32, tag="ssm_c")
                sqr_c = sm.tile([128, CH], f32, tag="sqr_c")
                rr_c = sm.tile([128, CH], f32, tag="rr_c")

                # row-norm sums of squares for the chunk, on DVE
                sqc = sm.tile([128, CH * 128], f32, tag="sqc", bufs=2)
                nc.vector.tensor_tensor(sqc[:, 0:ch * 128], m_all[:, msl],
                                        m_all[:, msl], ALU.mult)
                nc.vector.tensor_reduce(
                    ssm_c[:, 0:ch],
                    sqc[:, 0:ch * 128].rearrange("p (t u) -> p t u", u=UNITS),
                    mybir.AxisListType.X, ALU.add)

                # pass 1: write matmuls
                for t in range(ch):
                    tt = cb + t
                    m_t = m_all[:, tt * 128:(tt + 1) * 128]

                    wt16a = sm.tile([128, 128], bf16, tag="wt16a", bufs=2)
                    wt16b = sm.tile([128, 128], bf16, tag="wt16b", bufs=2)
                    pta = ps_tp.tile([128, 128], bf16, tag="tp")
                    nc.tensor.transpose(pta[:], cww16[:, t * 256:t * 256 + 128], id16[:])
                    nc.vector.tensor_copy(wt16a[:], pta[:])
                    ptb = ps_tp.tile([128, 128], bf16, tag="tp")
                    nc.tensor.transpose(ptb[:], cww16[:, t * 256 + 128:(t + 1) * 256], id16[:])
                    nc.vector.tensor_copy(wt16b[:], ptb[:])
                    w_ps = ps_w.tile([128, 128], f32, tag="wps")
                    nc.tensor.matmul(w_ps[:], lhsT=wt16a[:], rhs=h16[0][:],
                                     start=True, stop=False)
                    nc.tensor.matmul(w_ps[:], lhsT=wt16b[:], rhs=h16[1][:],
                                     start=False, stop=True)
                    nc.vector.tensor_copy(w_all[:, tt * 128:(tt + 1) * 128], w_ps[:])

                # sqrt + reciprocal of the row norms, batched per chunk
                nc.vector.tensor_scalar(ssm_c[:, 0:ch], ssm_c[:, 0:ch], 1e-12, None, ALU.max)
                nc.scalar.activation(sqr_c[:, 0:ch], ssm_c[:, 0:ch], AF.Sqrt)
                nc.vector.reciprocal(rr_c[:, 0:ch], sqr_c[:, 0:ch])

                # pass 2: transpose m, cosine matmul, softmax
                for t in range(ch):
                    tt = cb + t
                    m_t = m_all[:, tt * 128:(tt + 1) * 128]
                    bsl = slice(t * 256, (t + 1) * 256)

                    ptm = ps_tp.tile([128, 128], f32, tag="tp")
                    nc.tensor.transpose(ptm[:], m_t, id_t[:])
                    mT = sm.tile([128, 128], f32r, tag="mT")
                    nc.scalar.activation(mT[:], ptm[:], AF.Copy)

                    mc = ps_mc.tile([128, 256], f32, tag="mc")
                    nc.tensor.matmul(mc[:], lhsT=mT[:], rhs=nkey[:],
                                     start=True, stop=True)

                    et = sm.tile([128, 256], f32, tag="exp", bufs=2)
                    se = sm.tile([128, 1], f32, tag="se")
                    nc.scalar.activation(et[:], mc[:], AF.Exp,
                                         scale=rr_c[:, t:t + 1], accum_out=se[:])
                    rse = sm.tile([128, 1], f32, tag="rse")
                    nc.vector.reciprocal(rse[:], se[:])
                    nc.vector.tensor_scalar(cwr_out[:, bsl], et[:], rse[:], None,
                                            ALU.mult)

                # read matmuls (fp32)
                for t in range(ch):
                    tt = cb + t
                    nc.tensor.matmul(read_a[:], lhsT=cwr_out[:, t * 256:t * 256 + 128],
                                     rhs=m_all[:, tt * 128:(tt + 1) * 128],
                                     start=(tt == 0), stop=(tt == T - 1))
                    nc.tensor.matmul(read_b[:], lhsT=cwr_out[:, t * 256 + 128:(t + 1) * 256],
                                     rhs=m_all[:, tt * 128:(tt + 1) * 128],
                                     start=(tt == 0), stop=(tt == T - 1))

                # c_wu chunk: 0.95*c_wu_tm1 + c_wr + c_ww  (in place in cwu_in)
                nc.vector.tensor_scalar(cwu_in[:, 0:ch * 256], cwu_in[:, 0:ch * 256],
                                        USAGE_DECAY, None, ALU.mult)
                nc.vector.tensor_tensor(cwu_in[:, 0:ch * 256], cwu_in[:, 0:ch * 256],
                                        cwr_out[:, 0:ch * 256], ALU.add)
                nc.vector.tensor_tensor(cwu_all[:, csl], cwu_in[:, 0:ch * 256],
                                        cww_out[:, 0:ch * 256], ALU.add)

                # running per-batch minimum: one strided reduce over the
                # chunk's tile axis, then fold into the running min
                cwuc_v = cwu_all[:, csl].rearrange("p (t b) -> p b t", b=BATCH)
                mtree = sm.tile([128, 256], f32, tag="mtree", bufs=1)
                nc.vector.tensor_reduce(mtree[:], cwuc_v, mybir.AxisListType.X,
                                        ALU.min)
                if ci == 0:
                    nc.vector.tensor_copy(min_run[:], mtree[:])
                else:
                    nc.vector.tensor_tensor(min_run[:], min_run[:], mtree[:],
                                            ALU.min)

                # chunk outputs
                nc.sync.dma_start(
                    o_cwr[rows, :].rearrange("(t p) b -> p t b", p=128),
                    cwr_out[:, 0:ch * 256].rearrange("p (t b) -> p t b", b=BATCH))
                nc.sync.dma_start(
                    o_cww[rows, :].rearrange("(t p) b -> p t b", p=128),
                    cww_out[:, 0:ch * 256].rearrange("p (t b) -> p t b", b=BATCH))
            # ---------------- global min via AllGather ----------------
            negmin = const.tile([128, 256], f32)
            nc.vector.tensor_scalar(negmin[:], min_run[:], -1.0, None, ALU.mult)
            negred = const.tile([128, 256], f32)
            nc.gpsimd.partition_all_reduce(negred[:], negmin[:], channels=128,
                                           reduce_op=_reduce_max())
            mn_in = dram.tile([1, 256], f32)
            mn_out = dram.tile([N_CORES, 256], f32, addr_space="Shared")
            nc.sync.dma_start(mn_in[:], negred[0:1, :])
            cc_ag = nc.gpsimd.collective_compute(
                "AllGather", mybir.AluOpType.bypass,
                replica_groups=[list(range(N_CORES))],
                ins=[mn_in[:].opt()], outs=[mn_out[:].opt()])
            # c_wu output writeback deferred to here: it reads the persistent
            # cwu_all buffer and fills the AllGather skew window with DMA work
            for ci in range(NCHUNK):
                rows = slice(ci * CH * 128, (ci + 1) * CH * 128)
                csl = slice(ci * CH * 256, (ci + 1) * CH * 256)
                nc.sync.dma_start(
                    o_cwu[rows, :].rearrange("(t p) b -> p t b", p=128),
                    cwu_all[:, csl].rearrange("p (t b) -> p t b", b=BATCH))
            negall = const.tile([N_CORES, 256], f32)
            nc.sync.dma_start(negall[:], mn_out[:])
            negall_r = const.tile([N_CORES, 256], f32)
            nc.gpsimd.partition_all_reduce(negall_r[:], negall[:], channels=N_CORES,
                                           reduce_op=_reduce_max())
            gmin_row = const.tile([1, 256], f32)
            nc.vector.tensor_scalar(gmin_row[:], negall_r[0:1, :], -1.0, None,
                                    ALU.mult)
            gmin_b = const.tile([128, 256], f32)
            nc.gpsimd.partition_broadcast(gmin_b[:], gmin_row[:])
            gmin_rep = gmin_b[:].rearrange("p (x b) -> p x b", x=1) \
                                .broadcast_to([128, CH, 256])

            # ---------------- phase C: c_wlu + memory write ----------------
            for ci in range(NCHUNK):
                rows = slice(ci * CH * 128, (ci + 1) * CH * 128)
                csl = slice(ci * CH * 256, (ci + 1) * CH * 256)
                msl = slice(ci * CH * 128, (ci + 1) * CH * 128)
                # reuse phase-A stream slots (those tags are dead by now)
                cwlu_out = stream.tile([128, CH * 256], f32, tag="cwu_in")
                memc = stream.tile([128, CH * 128], f32, tag="cwr_out")
                nc.vector.tensor_tensor(
                    cwlu_out[:].rearrange("p (t b) -> p t b", b=BATCH),
                    cwu_all[:, csl].rearrange("p (t b) -> p t b", b=BATCH),
                    gmin_rep, ALU.is_le)
                cnt_c = sm.tile([128, CH], f32, tag="cnt_c")
                nc.vector.tensor_reduce(
                    cnt_c[:], cwlu_out[:].rearrange("p (t b) -> p t b", b=BATCH),
                    mybir.AxisListType.X, ALU.add)
                scl_c = sm.tile([128, CH], f32, tag="scl_c")
                nc.vector.tensor_scalar(scl_c[:], cnt_c[:], -1.0, float(BATCH),
                                        ALU.mult, ALU.add)
                for t in range(CH):
                    tt = ci * CH + t
                    nc.scalar.activation(memc[:, t * 128:(t + 1) * 128],
                                         m_all[:, tt * 128:(tt + 1) * 128],
                                         AF.Identity, scale=scl_c[:, t:t + 1])
                nc.vector.tensor_tensor(memc[:], memc[:], w_all[:, msl], ALU.add)
                nc.sync.dma_start(
                    o_cwlu[rows, :].rearrange("(t p) b -> p t b", p=128),
                    cwlu_out[:].rearrange("p (t b) -> p t b", b=BATCH))
                nc.sync.dma_start(
                    o_mem[rows, :].rearrange("(t p) u -> p t u", p=128),
                    memc[:].rearrange("p (t u) -> p t u", u=UNITS))

            # ---------------- read partial all-reduce (off critical path) ----
            read_sb = const.tile([128, 256], f32)
            nc.vector.tensor_copy(read_sb[:, 0:128], read_a[:])
            nc.vector.tensor_copy(read_sb[:, 128:256], read_b[:])
            rd_in = dram.tile([BATCH, UNITS], f32)
            rd_out = dram.tile([BATCH, UNITS], f32, addr_space="Shared")
            nc.sync.dma_start(rd_in[0:128, :], read_sb[:, 0:128])
            nc.sync.dma_start(rd_in[128:256, :], read_sb[:, 128:256])
            cc_ar = nc.gpsimd.collective_compute(
                "AllReduce", mybir.AluOpType.add,
                replica_groups=[list(range(N_CORES))],
                ins=[rd_in[:].opt()], outs=[rd_out[:].opt()])
            from concourse.tile_rust import add_dep_helper
            add_dep_helper(cc_ar.ins, cc_ag.ins, sync=True,
                           reason="AllGather feeds the critical path; run it first")
            nc.sync.dma_start(o_read[:], rd_out[:])

    nc.compile()
    return nc


def _reduce_max():
    from concourse import bass_isa
    return bass_isa.ReduceOp.max


def _ensure_built():
    if "nc" not in _state:
        _state["nc"] = _build()
    return _state["nc"]


def kernel(**inputs):
    from concourse import bass_utils

    nc = _ensure_built()

    inp = {k: np.asarray(v, dtype=np.float32) for k, v in inputs.items()}
    wg = 1.0 / (1.0 + np.exp(-inp["write_gate"].astype(np.float64)))
    wg32 = wg.astype(np.float32).reshape(1, 1)
    omw32 = (1.0 - wg32).astype(np.float32)

    shared = {
        "inputs": inp["inputs"],
        "r_tm1": inp["r_tm1"],
        "h_tm1": inp["h_tm1"],
        "c_tm1": inp["c_tm1"],
        "kern": inp["kernel"],
        "rec_kern": inp["rec_kernel"],
        "bias": inp["bias"].reshape(1, 4 * UNITS),
        "wg": wg32,
        "omw": omw32,
        "ident": np.eye(128, dtype=np.float32),
    }
    in_maps = []
    for c in range(N_CORES):
        rows = slice(c * SHARD, (c + 1) * SHARD)
        m = dict(shared)
        m["m_shard"] = np.ascontiguousarray(inp["m_tm1"][rows])
        m["c_wu_tm1"] = np.ascontiguousarray(inp["c_wu_tm1"][rows])
        m["c_wlu_tm1"] = np.ascontiguousarray(inp["c_wlu_tm1"][rows])
        m["c_wr_tm1"] = np.ascontiguousarray(inp["c_wr_tm1"][rows])
        in_maps.append(m)

    res = bass_utils.run_bass_kernel_spmd(
        nc, in_maps, core_ids=list(range(N_CORES)),
        trace=bool(_state.get("trace", False)))
    _state["last_result"] = res
    r = res.results

    read = r[0]["o_read"]
    h = r[0]["o_h"]
    c = r[0]["o_c"]
    memory = np.concatenate([r[i]["o_mem"] for i in range(N_CORES)], axis=0)
    c_wu = np.concatenate([r[i]["o_cwu"] for i in range(N_CORES)], axis=0)
    c_wlu = np.concatenate([r[i]["o_cwlu"] for i in range(N_CORES)], axis=0)
    c_wr = np.concatenate([r[i]["o_cwr"] for i in range(N_CORES)], axis=0)
    c_ww = np.concatenate([r[i]["o_cww"] for i in range(N_CORES)], axis=0)
    return read, memory, c_wu, c_wlu, c_wr, c_ww, h, c


# revision 31
# speedup vs baseline: 1.2334x; 1.0718x over previous
"""MANN LSTMCell step (scatter_memory) on 8 Trainium2 NeuronCores.

Sharding: the 32768-row memory axis is split 4096 rows/core (softmax over the
batch axis is per-mem-row, so it stays local; the memory write needs no
all-reduce in this decomposition).  Cross-core communication is one tiny
AllGather (per-batch local minima of the usage matrix, 1 KB) and one 128 KB
AllReduce (partial read vectors).  The LSTM controller is replicated on every
core.

The cosine-similarity matmul runs in fp32 (its result feeds an argmin whose
safety margin is ~4e-6); the read and write matmuls run in bf16 (their
outputs have orders-of-magnitude looser tolerances).
"""
import sys
import numpy as np

sys.path.insert(0, '/opt/trn_rl_repo')

MEM, UNITS, BATCH, IN_DIM = 32768, 128, 256, 512
N_CORES = 8
SHARD = MEM // N_CORES          # 4096 mem rows per core
T = SHARD // 128                # 32 tiles of 128 rows
CH = 4                          # tiles per DMA chunk (512 KB chunks)
NCHUNK = T // CH
USAGE_DECAY = 0.95

_state = {}


def _build():
    import concourse.bass as bass
    import concourse.bacc as bacc
    import concourse.mybir as mybir
    import concourse.tile as tile

    f32 = mybir.dt.float32
    bf16 = mybir.dt.bfloat16
    AF = mybir.ActivationFunctionType
    ALU = mybir.AluOpType

    nc = bacc.Bacc("TRN2", target_bir_lowering=False, debug=False,
                   num_devices=N_CORES)

    def din(name, shape):
        return nc.dram_tensor(name, shape, f32, kind="ExternalInput").ap()

    def dout(name, shape):
        return nc.dram_tensor(name, shape, f32, kind="ExternalOutput").ap()

    inputs_d = din("inputs", [BATCH, IN_DIM])
    r_d = din("r_tm1", [BATCH, UNITS])
    h_d = din("h_tm1", [BATCH, UNITS])
    c_d = din("c_tm1", [BATCH, UNITS])
    k_d = din("kern", [IN_DIM + UNITS, 4 * UNITS])
    rk_d = din("rec_kern", [UNITS, 4 * UNITS])
    b_d = din("bias", [1, 4 * UNITS])
    wg_d = din("wg", [1, 1])
    omw_d = din("omw", [1, 1])
    ident_d = din("ident", [128, 128])
    m_d = din("m_shard", [SHARD, UNITS])
    cwu0_d = din("c_wu_tm1", [SHARD, BATCH])
    cwlu0_d = din("c_wlu_tm1", [SHARD, BATCH])
    cwr0_d = din("c_wr_tm1", [SHARD, BATCH])

    o_cwr = dout("o_cwr", [SHARD, BATCH])
    o_cww = dout("o_cww", [SHARD, BATCH])
    o_cwu = dout("o_cwu", [SHARD, BATCH])
    o_cwlu = dout("o_cwlu", [SHARD, BATCH])
    o_mem = dout("o_mem", [SHARD, UNITS])
    o_read = dout("o_read", [BATCH, UNITS])
    o_h = dout("o_h", [BATCH, UNITS])
    o_c = dout("o_c", [BATCH, UNITS])

    with tile.TileContext(nc) as tc:
        with tc.tile_pool(name="const", bufs=1) as const, \
             tc.tile_pool(name="big", bufs=1) as big, \
             tc.tile_pool(name="wts", bufs=1) as wts, \
             tc.tile_pool(name="stream", bufs=3) as stream, \
             tc.tile_pool(name="sm", bufs=3) as sm, \
             tc.tile_pool(name="ps_read", bufs=1, space="PSUM") as ps_read, \
             tc.tile_pool(name="ps_mc", bufs=2, space="PSUM") as ps_mc, \
             tc.tile_pool(name="ps_tp", bufs=2, space="PSUM") as ps_tp, \
             tc.tile_pool(name="ps_w", bufs=2, space="PSUM") as ps_w, \
             tc.tile_pool(name="dram", bufs=1, space="DRAM") as dram:

            # ---------------- constants ----------------
            id_t = const.tile([128, 128], f32)
            nc.sync.dma_start(id_t[:], ident_d[:])
            id16 = const.tile([128, 128], bf16)
            nc.vector.tensor_copy(id16[:], id_t[:])
            id95 = const.tile([128, 128], f32)
            nc.vector.tensor_scalar(id95[:], id_t[:], USAGE_DECAY, None, ALU.mult)

            wg_raw = const.tile([1, 1], f32)
            omw_raw = const.tile([1, 1], f32)
            nc.sync.dma_start(wg_raw[:], wg_d[:])
            nc.sync.dma_start(omw_raw[:], omw_d[:])
            wg_b = const.tile([128, 1], f32)
            omw_b = const.tile([128, 1], f32)
            nc.gpsimd.partition_broadcast(wg_b[:], wg_raw[:])
            nc.gpsimd.partition_broadcast(omw_b[:], omw_raw[:])

            bias_row = const.tile([1, 512], f32)
            nc.sync.dma_start(bias_row[:], b_d[:])
            bias_b = const.tile([128, 512], f32)
            nc.gpsimd.partition_broadcast(bias_b[:], bias_row[:])

            # ---------------- LSTM controller (replicated) ----------------
            inp_sb = []
            r_sb = []
            h_sb = []
            c_sb = []
            for bh in range(2):
                t_in = wts.tile([128, 512], f32, name=f"inp{bh}")
                nc.sync.dma_start(t_in[:], inputs_d[bh * 128:(bh + 1) * 128, :])
                inp_sb.append(t_in)
                t_r = wts.tile([128, 128], f32, name=f"r{bh}")
                nc.sync.dma_start(t_r[:], r_d[bh * 128:(bh + 1) * 128, :])
                r_sb.append(t_r)
                t_h = wts.tile([128, 128], f32, name=f"h{bh}")
                nc.sync.dma_start(t_h[:], h_d[bh * 128:(bh + 1) * 128, :])
                h_sb.append(t_h)
                t_c = wts.tile([128, 128], f32, name=f"c{bh}")
                nc.sync.dma_start(t_c[:], c_d[bh * 128:(bh + 1) * 128, :])
                c_sb.append(t_c)

            k_sb = []
            for kb in range(5):
                t_k = wts.tile([128, 512], f32, name=f"k{kb}")
                nc.sync.dma_start(t_k[:], k_d[kb * 128:(kb + 1) * 128, :])
                k_sb.append(t_k)
            rk_sb = wts.tile([128, 512], f32)
            nc.sync.dma_start(rk_sb[:], rk_d[:])

            # transposed concat([inputs, r], -1): ciT[kb] is [128k, 256b]
            ciT = [wts.tile([128, 256], f32, name=f"ciT{kb}") for kb in range(5)]
            hT = wts.tile([128, 256], f32)
            for bh in range(2):
                for fb in range(4):
                    pt = ps_tp.tile([128, 128], f32, tag="tp")
                    nc.tensor.transpose(pt[:], inp_sb[bh][:, fb * 128:(fb + 1) * 128], id_t[:])
                    nc.scalar.activation(ciT[fb][:, bh * 128:(bh + 1) * 128], pt[:], AF.Copy)
                pt = ps_tp.tile([128, 128], f32, tag="tp")
                nc.tensor.transpose(pt[:], r_sb[bh][:], id_t[:])
                nc.scalar.activation(ciT[4][:, bh * 128:(bh + 1) * 128], pt[:], AF.Copy)
                pt = ps_tp.tile([128, 128], f32, tag="tp")
                nc.tensor.transpose(pt[:], h_sb[bh][:], id_t[:])
                nc.scalar.activation(hT[:, bh * 128:(bh + 1) * 128], pt[:], AF.Copy)

            h_new = []
            h16 = []
            f32r = mybir.dt.float32r
            nkey = const.tile([128, 256], f32r)
            for bh in range(2):
                zp = ps_mc.tile([128, 512], f32, tag="mc")
                for kb in range(5):
                    nc.tensor.matmul(zp[:], lhsT=ciT[kb][:, bh * 128:(bh + 1) * 128],
                                     rhs=k_sb[kb][:], start=(kb == 0), stop=False)
                nc.tensor.matmul(zp[:], lhsT=hT[:, bh * 128:(bh + 1) * 128],
                                 rhs=rk_sb[:], start=False, stop=True)
                z_sb = wts.tile([128, 512], f32, name=f"z{bh}", tag=f"inp{bh}")
                nc.vector.tensor_tensor(z_sb[:], zp[:], bias_b[:], ALU.add)

                ig = sm.tile([128, 128], f32, bufs=1)
                fg = sm.tile([128, 128], f32, bufs=1)
                gg = sm.tile([128, 128], f32, bufs=1)
                og = sm.tile([128, 128], f32, bufs=1)
                nc.scalar.activation(ig[:], z_sb[:, 0:128], AF.Sigmoid)
                nc.scalar.activation(fg[:], z_sb[:, 128:256], AF.Sigmoid)
                nc.scalar.activation(gg[:], z_sb[:, 256:384], AF.Tanh)
                nc.scalar.activation(og[:], z_sb[:, 384:512], AF.Sigmoid)
                t_ig = sm.tile([128, 128], f32, bufs=1)
                nc.vector.tensor_tensor(t_ig[:], ig[:], gg[:], ALU.mult)
                t_fc = sm.tile([128, 128], f32, bufs=1)
                nc.vector.tensor_tensor(t_fc[:], fg[:], c_sb[bh][:], ALU.mult)
                cn = const.tile([128, 128], f32, name=f"cnew{bh}")
                nc.vector.tensor_tensor(cn[:], t_fc[:], t_ig[:], ALU.add)
                tc_ = sm.tile([128, 128], f32, bufs=1)
                nc.scalar.activation(tc_[:], cn[:], AF.Tanh)
                hn = const.tile([128, 128], f32, name=f"hnew{bh}")
                nc.vector.tensor_tensor(hn[:], og[:], tc_[:], ALU.mult)
                h_new.append(hn)
                nc.sync.dma_start(o_h[bh * 128:(bh + 1) * 128, :], hn[:])
                nc.sync.dma_start(o_c[bh * 128:(bh + 1) * 128, :], cn[:])
                hb = const.tile([128, 128], bf16, name=f"h16_{bh}")
                nc.vector.tensor_copy(hb[:], hn[:])
                h16.append(hb)

                # l2-normalize rows of h (= columns of key_list)
                sq_s = sm.tile([128, 128], f32, bufs=1)
                ss = sm.tile([128, 1], f32)
                nc.scalar.activation(sq_s[:], hn[:], AF.Square, accum_out=ss[:])
                ssm = sm.tile([128, 1], f32)
                nc.vector.tensor_scalar(ssm[:], ss[:], 1e-12, None, ALU.max)
                sq2 = sm.tile([128, 1], f32)
                nc.scalar.activation(sq2[:], ssm[:], AF.Sqrt)
                rr = sm.tile([128, 1], f32)
                nc.vector.reciprocal(rr[:], sq2[:])
                nh = sm.tile([128, 128], f32, bufs=1)
                nc.vector.tensor_scalar(nh[:], hn[:], rr[:], None, ALU.mult)
                pt = ps_tp.tile([128, 128], f32, tag="tp")
                nc.tensor.transpose(pt[:], nh[:], id_t[:])
                nc.scalar.activation(nkey[:, bh * 128:(bh + 1) * 128], pt[:], AF.Copy)

            # ---------------- persistent big buffers ----------------
            m_all = big.tile([128, T * 128], f32)
            cwu_all = big.tile([128, T * 256], f32)
            w_all = big.tile([128, T * 128], f32)
            min_run = const.tile([128, 256], f32)

            read_a = ps_read.tile([128, 128], f32)
            read_b = ps_read.tile([128, 128], f32)

            # ---------------- phase A: main streaming loop ----------------
            # tapered: the last chunks shrink so the local min (and with it
            # the AllGather) launches as early as possible
            chunk_plan = []
            base = 0
            for ch in (4, 4, 4, 4, 4, 4, 4, 2, 2):
                chunk_plan.append((base, ch))
                base += ch
            assert base == T
            for ci, (cb, ch) in enumerate(chunk_plan):
                rows = slice(cb * 128, (cb + ch) * 128)
                csl = slice(cb * 256, (cb + ch) * 256)
                msl = slice(cb * 128, (cb + ch) * 128)
                cwu_in = stream.tile([128, CH * 256], f32, tag="cwu_in")
                cwr_in = stream.tile([128, CH * 256], f32, tag="cwr_in")
                cwlu_in = stream.tile([128, CH * 256], f32, tag="cwlu_in")
                nc.sync.dma_start(
                    cwu_in[:, 0:ch * 256].rearrange("p (t b) -> p t b", b=BATCH),
                    cwu0_d[rows, :].rearrange("(t p) b -> p t b", p=128))
                nc.sync.dma_start(
                    cwr_in[:, 0:ch * 256].rearrange("p (t b) -> p t b", b=BATCH),
                    cwr0_d[rows, :].rearrange("(t p) b -> p t b", p=128))
                nc.sync.dma_start(
                    cwlu_in[:, 0:ch * 256].rearrange("p (t b) -> p t b", b=BATCH),
                    cwlu0_d[rows, :].rearrange("(t p) b -> p t b", p=128))
                nc.sync.dma_start(
                    m_all[:, msl].rearrange("p (t u) -> p t u", u=UNITS),
                    m_d[rows, :].rearrange("(t p) u -> p t u", p=128))

                cwr_out = stream.tile([128, CH * 256], f32, tag="cwr_out")
                cww_out = stream.tile([128, CH * 256], f32, tag="cww_out")

                # c_ww chunk: affine into cww_out, then += c_wlu_tm1 (in place)
                nc.vector.tensor_scalar(cww_out[:, 0:ch * 256], cwr_in[:, 0:ch * 256], wg_b[:], omw_b[:],
                                        ALU.mult, ALU.add)
                nc.vector.tensor_tensor(cww_out[:, 0:ch * 256], cww_out[:, 0:ch * 256], cwlu_in[:, 0:ch * 256], ALU.add)
                cww16 = stream.tile([128, CH * 256], bf16, tag="cww16", bufs=1)
                nc.vector.tensor_copy(cww16[:, 0:ch * 256], cww_out[:, 0:ch * 256])

                # batched row-norm stats for this chunk
                ssm_c = sm.tile([128, CH], f32, tag="ssm_c")
                sqr_c = sm.tile([128, CH], f32, tag="sqr_c")
                rr_c = sm.tile([128, CH], f32, tag="rr_c")

                # row-norm sums of squares for the chunk, on DVE
                sqc = sm.tile([128, CH * 128], f32, tag="sqc", bufs=2)
                nc.vector.tensor_tensor(sqc[:, 0:ch * 128], m_all[:, msl],
                                        m_all[:, msl], ALU.mult)
                nc.vector.tensor_reduce(
                    ssm_c[:, 0:ch],
                    sqc[:, 0:ch * 128].rearrange("p (t u) -> p t u", u=UNITS),
                    mybir.AxisListType.X, ALU.add)

                # pass 1: write matmuls
                for t in range(ch):
                    tt = cb + t
                    m_t = m_all[:, tt * 128:(tt + 1) * 128]

                    wt16a = sm.tile([128, 128], bf16, tag="wt16a", bufs=2)
                    wt16b = sm.tile([128, 128], bf16, tag="wt16b", bufs=2)
                    pta = ps_tp.tile([128, 128], bf16, tag="tp")
                    nc.tensor.transpose(pta[:], cww16[:, t * 256:t * 256 + 128], id16[:])
                    nc.scalar.activation(wt16a[:], pta[:], AF.Copy)
                    ptb = ps_tp.tile([128, 128], bf16, tag="tp")
                    nc.tensor.transpose(ptb[:], cww16[:, t * 256 + 128:(t + 1) * 256], id16[:])
                    nc.vector.tensor_copy(wt16b[:], ptb[:])
                    w_ps = ps_w.tile([128, 128], f32, tag="wps")
                    nc.tensor.matmul(w_ps[:], lhsT=wt16a[:], rhs=h16[0][:],
                                     start=True, stop=False)
                    nc.tensor.matmul(w_ps[:], lhsT=wt16b[:], rhs=h16[1][:],
                                     start=False, stop=True)
                    nc.scalar.activation(w_all[:, tt * 128:(tt + 1) * 128], w_ps[:],
                                         AF.Copy)

                # sqrt + reciprocal of the row norms, batched per chunk
                nc.vector.tensor_scalar(ssm_c[:, 0:ch], ssm_c[:, 0:ch], 1e-12, None, ALU.max)
                nc.scalar.activation(sqr_c[:, 0:ch], ssm_c[:, 0:ch], AF.Sqrt)
                nc.vector.reciprocal(rr_c[:, 0:ch], sqr_c[:, 0:ch])

                # pass 2: transpose m, cosine matmul, softmax
                for t in range(ch):
                    tt = cb + t
                    m_t = m_all[:, tt * 128:(tt + 1) * 128]
                    bsl = slice(t * 256, (t + 1) * 256)

                    ptm = ps_tp.tile([128, 128], f32, tag="tp")
                    nc.tensor.transpose(ptm[:], m_t, id_t[:])
                    mT = sm.tile([128, 128], f32r, tag="mT")
                    nc.scalar.activation(mT[:], ptm[:], AF.Copy)

                    mc = ps_mc.tile([128, 256], f32, tag="mc")
                    nc.tensor.matmul(mc[:], lhsT=mT[:], rhs=nkey[:],
                                     start=True, stop=True)

                    et = sm.tile([128, 256], f32, tag="exp", bufs=2)
                    se = sm.tile([128, 1], f32, tag="se")
                    nc.scalar.activation(et[:], mc[:], AF.Exp,
                                         scale=rr_c[:, t:t + 1], accum_out=se[:])
                    rse = sm.tile([128, 1], f32, tag="rse")
                    nc.vector.reciprocal(rse[:], se[:])
                    nc.vector.tensor_scalar(cwr_out[:, bsl], et[:], rse[:], None,
                                            ALU.mult)

                # read matmuls (fp32)
                for t in range(ch):
                    tt = cb + t
                    nc.tensor.matmul(read_a[:], lhsT=cwr_out[:, t * 256:t * 256 + 128],
                                     rhs=m_all[:, tt * 128:(tt + 1) * 128],
                                     start=(tt == 0), stop=(tt == T - 1))
                    nc.tensor.matmul(read_b[:], lhsT=cwr_out[:, t * 256 + 128:(t + 1) * 256],
                                     rhs=m_all[:, tt * 128:(tt + 1) * 128],
                                     start=(tt == 0), stop=(tt == T - 1))

                # c_wu chunk: 0.95*c_wu_tm1 + c_wr + c_ww  (in place in cwu_in)
                nc.vector.tensor_scalar(cwu_in[:, 0:ch * 256], cwu_in[:, 0:ch * 256],
                                        USAGE_DECAY, None, ALU.mult)
                nc.vector.tensor_tensor(cwu_in[:, 0:ch * 256], cwu_in[:, 0:ch * 256],
                                        cwr_out[:, 0:ch * 256], ALU.add)
                nc.vector.tensor_tensor(cwu_all[:, csl], cwu_in[:, 0:ch * 256],
                                        cww_out[:, 0:ch * 256], ALU.add)

                # running per-batch minimum: one strided reduce over the
                # chunk's tile axis, then fold into the running min
                cwuc_v = cwu_all[:, csl].rearrange("p (t b) -> p b t", b=BATCH)
                mtree = sm.tile([128, 256], f32, tag="mtree", bufs=1)
                nc.vector.tensor_reduce(mtree[:], cwuc_v, mybir.AxisListType.X,
                                        ALU.min)
                if ci == 0:
                    nc.vector.tensor_copy(min_run[:], mtree[:])
                else:
                    nc.vector.tensor_tensor(min_run[:], min_run[:], mtree[:],
                                            ALU.min)

                # chunk outputs
                nc.sync.dma_start(
                    o_cwr[rows, :].rearrange("(t p) b -> p t b", p=128),
                    cwr_out[:, 0:ch * 256].rearrange("p (t b) -> p t b", b=BATCH))
                nc.sync.dma_start(
                    o_cww[rows, :].rearrange("(t p) b -> p t b", p=128),
                    cww_out[:, 0:ch * 256].rearrange("p (t b) -> p t b", b=BATCH))
            # ---------------- global min via AllGather ----------------
            negmin = const.tile([128, 256], f32)
            nc.vector.tensor_scalar(negmin[:], min_run[:], -1.0, None, ALU.mult)
            negred = const.tile([128, 256], f32)
            par_i = nc.gpsimd.partition_all_reduce(negred[:], negmin[:], channels=128,
                                                   reduce_op=_reduce_max())
            mn_in = dram.tile([1, 256], f32)
            mn_out = dram.tile([N_CORES, 256], f32, addr_space="Shared")
            nc.sync.dma_start(mn_in[:], negred[0:1, :])
            cc_ag = nc.gpsimd.collective_compute(
                "AllGather", mybir.AluOpType.bypass,
                replica_groups=[list(range(N_CORES))],
                ins=[mn_in[:].opt()], outs=[mn_out[:].opt()])
            # c_wu output writeback deferred to here: it reads the persistent
            # cwu_all buffer and fills the AllGather skew window with DMA work
            from concourse.tile_rust import add_dep_helper as _adh
            for ci in range(NCHUNK):
                rows = slice(ci * CH * 128, (ci + 1) * CH * 128)
                csl = slice(ci * CH * 256, (ci + 1) * CH * 256)
                dd = nc.sync.dma_start(
                    o_cwu[rows, :].rearrange("(t p) b -> p t b", p=128),
                    cwu_all[:, csl].rearrange("p (t b) -> p t b", b=BATCH))
                _adh(dd.ins, par_i.ins, sync=True,
                     reason="hold c_wu writeback for the AllGather skew window")
            negall = const.tile([N_CORES, 256], f32)
            nc.sync.dma_start(negall[:], mn_out[:])
            negall_r = const.tile([N_CORES, 256], f32)
            nc.gpsimd.partition_all_reduce(negall_r[:], negall[:], channels=N_CORES,
                                           reduce_op=_reduce_max())
            gmin_row = const.tile([1, 256], f32)
            nc.vector.tensor_scalar(gmin_row[:], negall_r[0:1, :], -1.0, None,
                                    ALU.mult)
            gmin_b = const.tile([128, 256], f32)
            nc.gpsimd.partition_broadcast(gmin_b[:], gmin_row[:])
            gmin_rep = gmin_b[:].rearrange("p (x b) -> p x b", x=1) \
                                .broadcast_to([128, CH, 256])

            # ---------------- phase C: c_wlu + memory write ----------------
            for ci in range(NCHUNK):
                rows = slice(ci * CH * 128, (ci + 1) * CH * 128)
                csl = slice(ci * CH * 256, (ci + 1) * CH * 256)
                msl = slice(ci * CH * 128, (ci + 1) * CH * 128)
                # reuse phase-A stream slots (those tags are dead by now)
                cwlu_out = stream.tile([128, CH * 256], f32, tag="cwu_in")
                memc = stream.tile([128, CH * 128], f32, tag="cwr_out")
                nc.vector.tensor_tensor(
                    cwlu_out[:].rearrange("p (t b) -> p t b", b=BATCH),
                    cwu_all[:, csl].rearrange("p (t b) -> p t b", b=BATCH),
                    gmin_rep, ALU.is_le)
                cnt_c = sm.tile([128, CH], f32, tag="cnt_c")
                if ci % 2 == 0:
                    nc.vector.tensor_reduce(
                        cnt_c[:], cwlu_out[:].rearrange("p (t b) -> p t b", b=BATCH),
                        mybir.AxisListType.X, ALU.add)
                else:
                    for t in range(CH):
                        nc.scalar.activation(
                            cwlu_out[:, t * 256:(t + 1) * 256],
                            cwlu_out[:, t * 256:(t + 1) * 256],
                            AF.Identity, accum_out=cnt_c[:, t:t + 1])
                scl_c = sm.tile([128, CH], f32, tag="scl_c")
                nc.vector.tensor_scalar(scl_c[:], cnt_c[:], -1.0, float(BATCH),
                                        ALU.mult, ALU.add)
                for t in range(CH):
                    tt = ci * CH + t
                    nc.scalar.activation(memc[:, t * 128:(t + 1) * 128],
                                         m_all[:, tt * 128:(tt + 1) * 128],
                                         AF.Identity, scale=scl_c[:, t:t + 1])
                nc.vector.tensor_tensor(memc[:], memc[:], w_all[:, msl], ALU.add)
                nc.sync.dma_start(
                    o_cwlu[rows, :].rearrange("(t p) b -> p t b", p=128),
                    cwlu_out[:].rearrange("p (t b) -> p t b", b=BATCH))
                nc.sync.dma_start(
                    o_mem[rows, :].rearrange("(t p) u -> p t u", p=128),
                    memc[:].rearrange("p (t u) -> p t u", u=UNITS))

            # ---------------- read partial all-reduce (off critical path) ----
            read_sb = const.tile([128, 256], f32)
            nc.vector.tensor_copy(read_sb[:, 0:128], read_a[:])
            nc.vector.tensor_copy(read_sb[:, 128:256], read_b[:])
            rd_in = dram.tile([BATCH, UNITS], f32)
            rd_out = dram.tile([BATCH, UNITS], f32, addr_space="Shared")
            nc.sync.dma_start(rd_in[0:128, :], read_sb[:, 0:128])
            nc.sync.dma_start(rd_in[128:256, :], read_sb[:, 128:256])
            cc_ar = nc.gpsimd.collective_compute(
                "AllReduce", mybir.AluOpType.add,
                replica_groups=[list(range(N_CORES))],
                ins=[rd_in[:].opt()], outs=[rd_out[:].opt()])
            from concourse.tile_rust import add_dep_helper
            add_dep_helper(cc_ar.ins, cc_ag.ins, sync=True,
                           reason="AllGather feeds the critical path; run it first")
            nc.sync.dma_start(o_read[:], rd_out[:])

    nc.compile()
    return nc


def _reduce_max():
    from concourse import bass_isa
    return bass_isa.ReduceOp.max


def _ensure_built():
    if "nc" not in _state:
        _state["nc"] = _build()
    return _state["nc"]


def kernel(**inputs):
    from concourse import bass_utils

    nc = _ensure_built()

    inp = {k: np.asarray(v, dtype=np.float32) for k, v in inputs.items()}
    wg = 1.0 / (1.0 + np.exp(-inp["write_gate"].astype(np.float64)))
    wg32 = wg.astype(np.float32).reshape(1, 1)
    omw32 = (1.0 - wg32).astype(np.float32)

    shared = {
        "inputs": inp["inputs"],
        "r_tm1": inp["r_tm1"],
        "h_tm1": inp["h_tm1"],
        "c_tm1": inp["c_tm1"],
        "kern": inp["kernel"],
        "rec_kern": inp["rec_kernel"],
        "bias": inp["bias"].reshape(1, 4 * UNITS),
        "wg": wg32,
        "omw": omw32,
        "ident": np.eye(128, dtype=np.float32),
    }
    in_maps = []
    for c in range(N_CORES):
        rows = slice(c * SHARD, (c + 1) * SHARD)
        m = dict(shared)
        m["m_shard"] = np.ascontiguousarray(inp["m_tm1"][rows])
        m["c_wu_tm1"] = np.ascontiguousarray(inp["c_wu_tm1"][rows])
        m["c_wlu_tm1"] = np.ascontiguousarray(inp["c_wlu_tm1"][rows])
        m["c_wr_tm1"] = np.ascontiguousarray(inp["c_wr_tm1"][rows])
        in_maps.append(m)

    res = bass_utils.run_bass_kernel_spmd(
        nc, in_maps, core_ids=list(range(N_CORES)),
        trace=bool(_state.get("trace", False)))
    _state["last_result"] = res
    r = res.results

    read = r[0]["o_read"]
    h = r[0]["o_h"]
    c = r[0]["o_c"]
    memory = np.concatenate([r[i]["o_mem"] for i in range(N_CORES)], axis=0)
    c_wu = np.concatenate([r[i]["o_cwu"] for i in range(N_CORES)], axis=0)
    c_wlu = np.concatenate([r[i]["o_cwlu"] for i in range(N_CORES)], axis=0)
    c_wr = np.concatenate([r[i]["o_cwr"] for i in range(N_CORES)], axis=0)
    c_ww = np.concatenate([r[i]["o_cww"] for i in range(N_CORES)], axis=0)
    return read, memory, c_wu, c_wlu, c_wr, c_ww, h, c
